# revision 38
# baseline (speedup 1.0000x reference)
"""Multi-head attention (B=2, L=2048, D=1024, H=16) on 8 Trainium2 NeuronCores.

Sharding: tensor-parallel over heads. Core c owns heads 2c, 2c+1, i.e. rows
[128c, 128c+128) of Wq/Wk/Wv and columns [128c, 128c+128) of Wo. Each core
computes Q/K/V projections for its 128 channels, attention for its 2 heads
(both batches), and a partial transposed out-projection yT_c = (attnO_c @
Wo[:, sl].T).T emitted in bf16. The host sums the 8 partials, transposes,
and adds bo.

Key optimizations over the naive layout:
- Key-padding-mask compaction: masked keys contribute exp(-inf)=0 exactly,
  so the host gathers only unmasked key/value tokens (padded to 128) and the
  kernel sizes its K/V projection + attention loops to the compacted length.
  With the ~50% random mask this halves scores/PV/exp work. Pad slots get
  bias -30000 so they exp to 0.0 like the reference's masked keys.
- All PE operands are bf16 (1 cycle/row; fp32 moving operands cost 4).
  Accumulation stays fp32 in PSUM.
- V is projected directly into [token, channel] layout (contraction chunks
  of x as the stationary operand), avoiding separate PE transposes.
- Scores are computed transposed (S.T tiles [k,q]) so softmax P.T lands in
  SBUF ready to be the PV matmul's moving operand; softmax max-subtraction
  is skipped (scores ~N(0,1), fp32 exp cannot overflow) and row sums ride
  along as a ones-column in the PV stationary, appearing as psum row 64.
- The out-projection is computed transposed (yT [D, T]): stationary
  woT chunks [128e, 128d] give contraction depth 128 (vs 64 the other way),
  halving out-proj PE time, and the bf16 yT output halves output DMA.
- Attention is software-pipelined with lag 2 (scores for kt issued two
  steps ahead of the PV accumulation of kt) so the PE never waits on the
  ACT exp; projection/output units are interleaved at generator yields.
"""

import os
import sys

for _p in ("/opt/trn_rl_repo", os.path.expanduser("~/.axon_site/_ro/trn_rl_repo")):
    if os.path.isdir(_p) and _p not in sys.path:
        sys.path.insert(0, _p)

import numpy as np

B = 2
L = 2048
D = 1024
T = B * L            # 4096 query tokens
E = 128              # channels per core (2 heads x 64)
HD = 64              # head dim
N_CORES = 8
SCALE = 1.0 / 8.0    # 1/sqrt(HD)
MASK_BIAS = -30000.0

N_DC = D // 128      # 8 contraction chunks
N_QT = L // 512      # 4 query tiles per batch

_cached = {}


def _build_program(has_bq, has_bk, has_bv, nkt0, nkt1, reps=1):
    import concourse.bacc as bacc
    import concourse.mybir as mybir
    import concourse.tile as tile

    F32 = mybir.dt.float32
    BF16 = mybir.dt.bfloat16
    AF = mybir.ActivationFunctionType
    ALU = mybir.AluOpType

    n_kt = (nkt0, nkt1)
    NG = nkt0 + nkt1               # 128-token key groups, both batches
    TK = 128 * NG                  # compacted+padded key tokens
    TKP = 512 * ((TK + 511) // 512)  # staged to 512-token proj tiles
    N_KVT = TKP // 512             # k/v projection token tiles
    N_QTT = T // 512               # q projection token tiles (8)
    goff = (0, nkt0)               # group offset per batch

    def mm(out, lhsT, rhs, **kw):
        nc.tensor.matmul(out, lhsT, rhs, **kw)

    nc = bacc.Bacc("TRN2", target_bir_lowering=False, debug=False,
                   num_devices=N_CORES)

    qT = nc.dram_tensor("qT", [D, T], BF16, kind="ExternalInput").ap()
    kT = nc.dram_tensor("kT", [D, TKP], BF16, kind="ExternalInput").ap()
    vT = nc.dram_tensor("vT", [D, TKP], BF16, kind="ExternalInput").ap()
    # w inputs are pre-chunked on the host: w[p, dc*128+e] = W.T[dc*128+p, e]
    # so the load is a plain contiguous DMA (2KB/partition descriptors).
    wq = nc.dram_tensor("wq", [128, D], BF16, kind="ExternalInput").ap()
    wk = nc.dram_tensor("wk", [128, D], BF16, kind="ExternalInput").ap()
    wv = nc.dram_tensor("wv", [128, D], BF16, kind="ExternalInput").ap()
    wo = nc.dram_tensor("wo", [E, D], BF16, kind="ExternalInput").ap()
    mbd = nc.dram_tensor("mb", [128, NG], F32, kind="ExternalInput").ap()
    onesd = nc.dram_tensor("ones128", [128, 128], BF16, kind="ExternalInput").ap()
    bias_d = {}
    if has_bq:
        bias_d["q"] = nc.dram_tensor("bq", [128, 1], F32, kind="ExternalInput").ap()
    if has_bk:
        bias_d["k"] = nc.dram_tensor("bk", [128, 1], F32, kind="ExternalInput").ap()
    if has_bv:
        bias_d["v"] = nc.dram_tensor("bv", [64, 2], F32, kind="ExternalInput").ap()
    yd = nc.dram_tensor("y", [D, T], BF16, kind="ExternalOutput").ap()

    with tile.TileContext(nc) as tc:
        import contextlib
        with contextlib.ExitStack() as ctx:
            const = ctx.enter_context(tc.tile_pool(name="const", bufs=1))
            big = ctx.enter_context(tc.tile_pool(name="big", bufs=1))
            stg = ctx.enter_context(tc.tile_pool(name="stg", bufs=10))
            work = ctx.enter_context(tc.tile_pool(name="work", bufs=6))
            pt_pool = ctx.enter_context(tc.tile_pool(name="ptp", bufs=3))
            psum = ctx.enter_context(tc.tile_pool(name="psum", bufs=2, space="PSUM"))
            psst = ctx.enter_context(tc.tile_pool(name="psst", bufs=2, space="PSUM"))
            psacc = ctx.enter_context(tc.tile_pool(name="psacc", bufs=2, space="PSUM"))

            # ---- weights needed by the first projections (the remaining
            # constants are DMA'd inside rep 0 at scheduled points) ----
            # weight tiles (DMAs are emitted by the rep-0 startup sequence
            # below, in deadline order on a single queue)
            w_sb = {}
            w_src = {"k": wk, "v": wv, "q": wq}
            for nm in ("k", "v", "q"):
                w_sb[nm] = const.tile([128, D], BF16, name=f"w{nm}_sb")
            wo_sb = const.tile([128, D], BF16, name="wo_sb")
            mb_sb = const.tile([128, NG], F32, name="mb_sb")
            ones_sb = const.tile([128, 128], BF16, name="ones_sb")
            b_sb = {}
            for nm in bias_d:
                b_sb[nm] = const.tile([128, 2] if nm == "v" else [128, 1], F32,
                                      name=f"b{nm}_sb")

            # ---- persistent activations ----
            QT = big.tile([128, T], BF16, name="QT")     # [e, q-tok]
            KT = big.tile([128, TKP], BF16, name="KT")   # [e, k-tok]
            # V layout per 128-token group g: [V_headA(64) | 1 | V_headB(64) | 1]
            # partitions = k-token; the ones column rides along in the PV
            # matmul and produces the softmax denominator as psum row 64.
            V = big.tile([128, NG * 130], BF16, name="V")
            for g in range(NG):
                nc.gpsimd.memset(V[:, g * 130 + 64:g * 130 + 65], 1.0)
                nc.gpsimd.memset(V[:, g * 130 + 129:g * 130 + 130], 1.0)
            OT = big.tile([128, T], BF16, name="OT")     # [e(2 heads), q-tok]
            scr = big.tile([1, 8], F32, name="scr")      # ACT table preload out

            for rep in range(reps):
                srcs = {"q": qT, "k": kT, "v": vT}
                staged = {}

                # k/v token-column units: 512 wide, but cut at the first
                # 256-aligned boundary past batch 0 so batch-1's share of
                # that tile drops out of the startup-critical DMA chain;
                # the zero-padded tail is trimmed entirely.
                b0e = min(nkt0 * 128, TK)
                cuts = list(range(0, b0e, 512))
                c = min(-(-b0e // 256) * 256, TK)
                if c > cuts[-1]:
                    cuts.append(c)
                while cuts[-1] < TK:
                    cuts.append(min(cuts[-1] + 512, TK))
                KVU = [(cuts[i], cuts[i + 1] - cuts[i])
                       for i in range(len(cuts) - 1)]
                N_KVU = len(KVU)
                n_b0u = sum(1 for c0, cw in KVU if c0 < b0e)

                def unit_cols(nm, u):
                    return (u * 512, 512) if nm == "q" else KVU[u]

                def proj_dma(nm, tt):
                    # one strided DMA lands all 8 contraction chunks
                    # side-by-side: s[p, dc*cw + t] = src[dc*128 + p, tt*512 + t]
                    # ALL input staging goes on the ONE scalar HWDGE queue:
                    # the DMA device serves transfer requests FIFO and HWDGE
                    # round-robins descriptor-gen between queues, so a
                    # single queue is the only way to control global
                    # transfer order (SWDGE gen is ~1us/DMA on Pool and
                    # loses the FIFO race entirely).
                    c0, cw = unit_cols(nm, tt)
                    s = stg.tile([128, N_DC * cw], BF16,
                                 name=f"{rep}_stg_{nm}_{tt}", tag="stg")
                    nc.scalar.dma_start(
                        s.rearrange("p (c t) -> p c t", c=N_DC),
                        srcs[nm][:, c0:c0 + cw]
                        .rearrange("(c p) t -> p c t", p=128))
                    staged[(nm, tt)] = s

                def proj_mm(nm, tt):
                    s = staged.pop((nm, tt))
                    c0, cw = unit_cols(nm, tt)
                    if nm in ("q", "k"):
                        ps = psum.tile([128, 512], F32,
                                       name=f"{rep}_ps_{nm}_{tt}", tag="mm")
                        for dc in range(N_DC):
                            mm(ps[:, 0:cw],
                               w_sb[nm][:, dc * 128:(dc + 1) * 128],
                               s[:, dc * cw:(dc + 1) * cw],
                               start=(dc == 0), stop=(dc == N_DC - 1))
                        dst = (QT if nm == "q" else KT)[:, c0:c0 + cw]
                        with nc.allow_low_precision(reason="bf16 activations"):
                            if nm in b_sb:
                                nc.vector.tensor_scalar(dst, ps[:, 0:cw],
                                                        b_sb[nm][:, 0:1],
                                                        None, ALU.add)
                            else:
                                nc.vector.tensor_copy(dst, ps[:, 0:cw])
                    else:
                        # V^T directly: out[t, e] accumulated over d-chunks
                        # with the x chunk as the stationary operand.
                        ps = psum.tile([128, 512], F32,
                                       name=f"{rep}_ps_v_{tt}", tag="mm")
                        for si in range(cw // 128):
                            if c0 // 128 + si >= NG:
                                break
                            for dc in range(N_DC):
                                mm(ps[:, si * 128:(si + 1) * 128],
                                   s[:, dc * cw + si * 128:dc * cw + (si + 1) * 128],
                                   w_sb["v"][:, dc * 128:(dc + 1) * 128],
                                   start=(dc == 0), stop=(dc == N_DC - 1))
                        with nc.allow_low_precision(reason="bf16 activations"):
                            for si in range(cw // 128):
                                g = c0 // 128 + si
                                if g >= NG:
                                    break
                                nc.vector.tensor_copy(
                                    V[:, g * 130:g * 130 + 64],
                                    ps[:, si * 128:si * 128 + 64])
                                nc.vector.tensor_copy(
                                    V[:, g * 130 + 65:g * 130 + 129],
                                    ps[:, si * 128 + 64:si * 128 + 128])

                # Software-pipelined attention for one (batch, 512-query
                # tile): scores for k-tile kt are issued on the PE two steps
                # before the PV accumulation of kt, so the PE never stalls on
                # the ACT exp of the tile it just produced. Yields carry
                # (pe_ns_just_emitted, key_groups_needed_next, at_normalize)
                # so the emitter can account PE time, force the K/V
                # projection units the next scores depend on, and slot
                # fillers into the dependency gaps.
                def attn_steps(b, qt, qo=0, qw=512):
                    q0 = b * L + qt * 512 + qo
                    nkt = n_kt[b]
                    kcost = (qw * 852) // 512
                    ot = [psacc.tile([65, qw], F32,
                                     name=f"{rep}_ot{h}_{b}_{qt}_{qo}",
                                     tag="acc")
                          for h in range(2)]

                    def scores(kt):
                        g = goff[b] + kt
                        k0 = g * 128
                        s = psst.tile([128, 2 * qw], F32,
                                      name=f"{rep}_st_{b}_{qt}_{qo}_{kt}",
                                      tag="st")
                        for h in range(2):
                            mm(s[:, h * qw:(h + 1) * qw],
                               KT[h * 64:(h + 1) * 64, k0:k0 + 128],
                               QT[h * 64:(h + 1) * 64, q0:q0 + qw],
                               start=True, stop=True)
                        p = pt_pool.tile([128, 2 * qw], BF16,
                                         name=f"{rep}_pt_{b}_{qt}_{qo}_{kt}",
                                         tag="pt")
                        nc.scalar.activation(p[:], s[:], AF.Exp,
                                             bias=mb_sb[:, g:g + 1],
                                             scale=SCALE)
                        return p

                    def pv(kt, p, last):
                        g = goff[b] + kt
                        for h in range(2):
                            mm(ot[h][:],
                               V[:, g * 130 + 65 * h: g * 130 + 65 * (h + 1)],
                               p[:, h * qw:(h + 1) * qw],
                               start=(kt == 0), stop=last)

                    gg = goff[b]
                    yield (0, [gg, gg + min(1, nkt - 1)], False)
                    pq = [scores(0)]
                    if nkt > 1:
                        pq.append(scores(1))
                    seg = kcost
                    for kt in range(2, nkt):
                        if kt % 2 == 0:
                            need = [gg + kt, gg + min(kt + 1, nkt - 1)]
                            yield (seg, need, False)
                            seg = 0
                        pq.append(scores(kt))
                        pv(kt - 2, pq.pop(0), last=False)
                        seg += kcost
                    for i, p in enumerate(pq):
                        kt = nkt - len(pq) + i
                        pv(kt, p, last=(kt == nkt - 1))
                        seg += kcost // 2

                    # normalize: OT rows = [otA/rA ; otB/rB]; 1/r broadcast
                    # across the 64 head partitions with a K=1 ones matmul
                    # (compute engines cannot move data across partitions).
                    # The yield lets PE fillers run while the DVE computes
                    # the reciprocals, instead of stalling on the bc matmul.
                    rb = [work.tile([65, qw], BF16,
                                    name=f"{rep}_rb{h}_{b}_{qt}_{qo}",
                                    tag=f"rb{h}")
                          for h in range(2)]
                    with nc.allow_low_precision(reason="feeds bf16 matmul"):
                        nc.vector.reciprocal(rb[0][64:65, :], ot[0][64:65, :])
                        nc.vector.reciprocal(rb[1][64:65, :], ot[1][64:65, :])
                    yield (seg, [], True)
                    bc_sb = work.tile([64, 2 * qw], F32,
                                      name=f"{rep}_bcs_{b}_{qt}_{qo}",
                                      tag="bcs")
                    for h in range(2):
                        bc_ps = psum.tile([64, qw], F32,
                                          name=f"{rep}_bc{h}_{b}_{qt}_{qo}",
                                          tag="mm")
                        mm(bc_ps[:], ones_sb[64:65, 0:64], rb[h][64:65, :],
                           start=True, stop=True)
                        nc.vector.tensor_copy(bc_sb[:, h * qw:(h + 1) * qw],
                                              bc_ps[:])
                    for h in range(2):
                        dst = OT[h * 64:(h + 1) * 64, q0:q0 + qw]
                        with nc.allow_low_precision(reason="bf16 attn output"):
                            nc.vector.tensor_mul(
                                dst, ot[h][0:64, :],
                                bc_sb[:, h * qw:(h + 1) * qw])
                        if "v" in b_sb:
                            nc.vector.tensor_scalar(dst, dst,
                                                    b_sb["v"][0:64, h:h + 1],
                                                    None, ALU.add)

                def y_unit(dc, c0, cw, tail=False):
                    # yT tiles [128 d, <=512 t]: contraction over all 128
                    # channels in one pass with the woT chunk stationary.
                    # Adjacent token tiles share one output DMA (via the
                    # otherwise-idle Pool engine's SWDGE) to halve the
                    # fixed per-DMA descriptor-generation cost.
                    ys = work.tile([128, cw], BF16,
                                   name=f"{rep}_ys_{dc}_{c0}", tag="ys")
                    for j in range(0, cw, 512):
                        w = min(512, cw - j)
                        # in the tail the scores pool is free: its extra
                        # PSUM banks deepen the mm->copy->DMA rotation
                        yp = (psst if tail else psum).tile(
                            [128, w], F32, name=f"{rep}_yp_{dc}_{c0 + j}",
                            tag="st" if tail else "mm")
                        mm(yp[:], wo_sb[:, dc * 128:(dc + 1) * 128],
                           OT[:, c0 + j:c0 + j + w],
                           start=True, stop=True)
                        # during attention the ACT engine is exp-bound: keep
                        # copies on the DVE; in the tail both are free.
                        with nc.allow_low_precision(reason="bf16 out"):
                            if tail and (dc + j // 512) % 2 == 0:
                                nc.scalar.copy(ys[:, j:j + w], yp[:])
                            else:
                                nc.vector.tensor_copy(ys[:, j:j + w], yp[:])
                    eng = (nc.sync if dc % 2 else nc.scalar) if tail \
                        else nc.gpsimd
                    eng.dma_start(
                        yd[dc * 128:(dc + 1) * 128, c0:c0 + cw], ys[:])

                # ---- emission schedule ----
                # DMA transfers are serialized at HBM bandwidth, so the
                # issue order is the data arrival order; a static clock
                # estimate paces issues ~LEAD ns ahead of PE consumption
                # and gates optional filler matmuls on estimated arrival.
                STG_NS, LEAD = 2950.0, 9000.0
                est = {"dma": 0.0, "pe": 0.0}
                ready = {}

                def issue_dma(u):
                    proj_dma(*u)
                    _, cw = unit_cols(*u)
                    est["dma"] = max(est["dma"], est["pe"]) + (STG_NS * cw) / 512
                    ready[u] = est["dma"] + 500.0

                def mm_unit(u):
                    _, cw = unit_cols(*u)
                    est["pe"] = max(est["pe"], ready[u]) + (1707.0 * cw) / 512
                    proj_mm(*u)

                kv_all = [(nm, u) for u in range(N_KVU) for nm in ("k", "v")]
                pre = kv_all[:2 * n_b0u]
                rest = kv_all[2 * n_b0u:]
                # units in mm-emission order; DMA order (with the consts
                # woven in by deadline) is built separately below.
                dma_order = [("k", 0), ("v", 0), ("q", 0)] + pre[2:]
                dma_order += [("q", t) for t in range(1, min(5, N_QTT))]
                dma_order += rest
                dma_order += [("q", t) for t in range(5, N_QTT)]

                dma_pend = list(dma_order)
                mm_pend = list(dma_order)
                y_pend = []

                def covered_units(groups):
                    need = []
                    for g in groups:
                        x = g * 128
                        for u, (c0, cw) in enumerate(KVU):
                            if c0 <= x < c0 + cw:
                                for nm in ("k", "v"):
                                    un = (nm, u)
                                    if un in mm_pend and un not in need:
                                        need.append(un)
                    return need

                # startup DMA sequence, strictly in deadline order on the
                # single scalar queue: each weight/const lands just before
                # its first consumer, each stg tile as early as possible.
                np_pre = len(pre) + 1
                stg_pre, ci = dma_pend[:np_pre], 0

                def const_dma(dst, src):
                    if rep == 0:
                        nc.scalar.dma_start(dst[:], src[:])
                        est["dma"] += 200.0

                const_dma(w_sb["k"], w_src["k"])
                est["dma"] += 550.0
                issue_dma(stg_pre[0])                     # k0
                const_dma(w_sb["v"], w_src["v"])
                est["dma"] += 550.0
                issue_dma(stg_pre[1])                     # v0
                issue_dma(stg_pre[2])                     # q0
                const_dma(mb_sb, mbd)
                for nm, bt in b_sb.items():
                    const_dma(bt, bias_d[nm])
                const_dma(w_sb["q"], w_src["q"])
                est["dma"] += 550.0
                if len(stg_pre) > 3:
                    issue_dma(stg_pre[3])                 # k1
                const_dma(ones_sb, onesd)
                for u in stg_pre[4:]:
                    issue_dma(u)                          # v1, k2, v2, ...
                del dma_pend[:np_pre]
                # load the ACT exp table during the idle startup window
                if rep == 0:
                    nc.scalar.activation(scr[0:1, 0:1], mb_sb[0:1, 0:1],
                                         AF.Exp, scale=1.0)
                for u in [("k", 0), ("v", 0), ("q", 0)]:
                    mm_unit(u)
                    mm_pend.remove(u)
                const_dma(wo_sb, wo)

                tiles = [(b, qt) for b in range(B) for qt in range(N_QT)]
                last_ti = len(tiles) - 1
                # the last tile runs as two half-width query windows so its
                # normalize + output drain overlaps the second half's
                # attention instead of sitting entirely in the tail.
                HALVE_LAST = False
                subtiles = []
                for ti in range(len(tiles)):
                    b, qt = tiles[ti]
                    if ti == last_ti and HALVE_LAST:
                        subtiles.append((ti, b, qt, 0, 256))
                        subtiles.append((ti, b, qt, 256, 256))
                    else:
                        subtiles.append((ti, b, qt, 0, 512))

                def run_subtile(ti, b, qt, qo, qw, final):
                    for cost, need, at_norm in attn_steps(b, qt, qo, qw):
                        est["pe"] += cost
                        while dma_pend and est["dma"] < est["pe"] + LEAD:
                            issue_dma(dma_pend.pop(0))
                        budget = 1800.0
                        # forced: K/V units the next scores depend on, and
                        # the next tile's q projection by its deadline.
                        forced = covered_units(need)
                        if at_norm and ("q", ti + 1) in mm_pend:
                            forced.append(("q", ti + 1))
                        for u in forced:
                            while u in dma_pend:  # must be issued by now
                                issue_dma(dma_pend.pop(0))
                            mm_unit(u)
                            mm_pend.remove(u)
                            budget -= 1707
                        if at_norm and final:
                            # final normalize: nothing else will fill the
                            # PE while the DVE reciprocals run - drain all
                            # ready y units here (copies on ACT: exp done)
                            for dc, c0, cw in y_pend:
                                y_unit(dc, c0, cw, tail=True)
                            del y_pend[:]
                            continue
                        # optional fillers: proj mms whose data has landed,
                        # then ready y units (max 2: deeper bursts stall on
                        # the 2-buffer PSUM pool rotation).
                        y_n = 0
                        while budget > 0:
                            pick = None
                            for u in mm_pend[:2]:
                                if (u in ready
                                        and ready[u] <= est["pe"] + 400
                                        and budget >= 1707):
                                    pick = u
                                    break
                            if pick is not None:
                                mm_unit(pick)
                                mm_pend.remove(pick)
                                budget -= 1707
                            elif (y_pend
                                    and y_n < 2
                                    and budget >= (250 * y_pend[0][2]) // 512):
                                dc, c0, cw = y_pend.pop(0)
                                y_unit(dc, c0, cw)
                                cost_y = (250 * cw) // 512
                                est["pe"] += cost_y
                                budget -= cost_y
                                y_n += 1
                            else:
                                break

                for si, (ti, b, qt, qo, qw) in enumerate(subtiles):
                    if qo == 0:
                        assert ("q", ti) not in mm_pend, f"q{ti} not emitted"
                    run_subtile(ti, b, qt, qo, qw,
                                final=(si == len(subtiles) - 1))
                    # this query window is normalized -> its y units are
                    # ready (full tiles pair up across odd ti for one DMA)
                    c0 = ti * 512 + qo
                    if qw == 512:
                        if ti % 2 == 1 and ti < 6:
                            y_pend.extend((dc, c0 - 512, 1024)
                                          for dc in range(N_DC))
                        elif ti >= 6:
                            y_pend.extend((dc, c0, 512)
                                          for dc in range(N_DC))
                    else:
                        y_pend.extend((dc, c0, qw) for dc in range(N_DC))
                for u in list(mm_pend):
                    while u in dma_pend:
                        issue_dma(dma_pend.pop(0))
                    mm_unit(u)
                    mm_pend.remove(u)
                for dc, c0, cw in y_pend:
                    y_unit(dc, c0, cw, tail=True)

    nc.compile()
    return nc


def _host_prep(q, k, v, mask, Wq, bq, Wk, bk, Wv, bv, Wo):
    """Build the per-core input maps. Compacts masked keys out of k/v."""
    import ml_dtypes
    f32 = np.float32
    bf16 = ml_dtypes.bfloat16

    qT = np.ascontiguousarray(q.reshape(T, D).T.astype(bf16))

    # --- key compaction: keep only unmasked tokens, pad groups to 128 ---
    idxs, biases, nkts = [], [], []
    for b in range(B):
        idx = np.flatnonzero(~mask[b])
        nkt = max(1, (len(idx) + 127) // 128)
        pad = 128 * nkt - len(idx)
        bias = np.concatenate([np.zeros(len(idx), f32),
                               np.full(pad, MASK_BIAS, f32)])
        idx = np.concatenate([idx, np.zeros(pad, np.int64)])
        idxs.append(idx)
        biases.append(bias)
        nkts.append(nkt)
    NG = sum(nkts)
    TK = 128 * NG
    TKP = 512 * ((TK + 511) // 512)
    kc = np.concatenate([k[b][idxs[b]] for b in range(B)], axis=0)
    vc = np.concatenate([v[b][idxs[b]] for b in range(B)], axis=0)
    kc = np.concatenate([kc, np.zeros((TKP - TK, D), kc.dtype)], axis=0)
    vc = np.concatenate([vc, np.zeros((TKP - TK, D), vc.dtype)], axis=0)
    kT = np.ascontiguousarray(kc.T.astype(bf16))
    vT = np.ascontiguousarray(vc.T.astype(bf16))
    mb = np.concatenate(biases).reshape(NG, 128).T
    mb = np.ascontiguousarray(mb.astype(f32))
    ones128 = np.ones((128, 128), bf16)

    def chunked(wT):
        # [D, E] -> [128, N_DC*E]: w[p, dc*E + e] = wT[dc*128 + p, e]
        return np.ascontiguousarray(
            wT.reshape(N_DC, 128, E).transpose(1, 0, 2).reshape(128, D))

    in_maps = []
    for c in range(N_CORES):
        sl = slice(c * E, (c + 1) * E)
        m = {
            "qT": qT, "kT": kT, "vT": vT,
            "wq": chunked(Wq[sl, :].T.astype(bf16)),
            "wk": chunked(Wk[sl, :].T.astype(bf16)),
            "wv": chunked(Wv[sl, :].T.astype(bf16)),
            "wo": np.ascontiguousarray(Wo[:, sl].T.astype(bf16)),
            "mb": mb, "ones128": ones128,
        }
        if np.any(bq):
            m["bq"] = np.ascontiguousarray(bq[sl].astype(f32).reshape(128, 1))
        if np.any(bk):
            m["bk"] = np.ascontiguousarray(bk[sl].astype(f32).reshape(128, 1))
        if np.any(bv):
            m["bv"] = np.ascontiguousarray(bv[sl].astype(f32).reshape(2, 64).T)
        in_maps.append(m)
    return in_maps, (nkts[0], nkts[1])


def _make_timed_runner(nc, in_maps):
    """Build a reusable jitted runner for `nc` (no output donation — the
    program writes every output element, so uninit result buffers are fine).
    Returns (run_once() -> per-core outputs as numpy, time_iters(n) -> [sec])."""
    import jax
    import time
    import concourse.mybir as mybir
    from concourse import bass2jax
    from jax.experimental.shard_map import shard_map
    from jax.sharding import Mesh, NamedSharding, PartitionSpec

    bass2jax.install_neuronx_cc_hook()

    partition_name = nc.partition_id_tensor.name if nc.partition_id_tensor else None
    in_names, out_names, out_avals, zero_outs = [], [], [], []
    for alloc in nc.m.functions[0].allocations:
        if not isinstance(alloc, mybir.MemoryLocationSet):
            continue
        name = alloc.memorylocations[0].name
        if alloc.kind == "ExternalInput":
            if name != partition_name:
                in_names.append(name)
        elif alloc.kind == "ExternalOutput":
            shape = tuple(alloc.tensor_shape)
            dtype = mybir.dt.np(alloc.dtype)
            out_names.append(name)
            out_avals.append(jax.core.ShapedArray(shape, dtype))
            zero_outs.append(np.zeros(shape, dtype))
    n_params = len(in_names)
    all_in_names = list(in_names) + list(out_names)
    if partition_name is not None:
        all_in_names.append(partition_name)

    def _body(*args):
        operands = list(args)
        if partition_name is not None:
            operands.append(bass2jax.partition_id_tensor())
        outs = bass2jax._bass_exec_p.bind(
            *operands,
            out_avals=tuple(out_avals),
            in_names=tuple(all_in_names),
            out_names=tuple(out_names),
            lowering_input_output_aliases=(),
            sim_require_finite=True,
            sim_require_nnan=True,
            nc=nc,
        )
        return tuple(outs)

    devices = jax.devices()[:N_CORES]
    mesh = Mesh(np.asarray(devices), ("core",))
    nin = n_params + len(out_names)
    fn = jax.jit(shard_map(_body, mesh=mesh,
                           in_specs=(PartitionSpec("core"),) * nin,
                           out_specs=(PartitionSpec("core"),) * len(out_names),
                           check_rep=False))
    sh = NamedSharding(mesh, PartitionSpec("core"))
    dev_args = [
        jax.device_put(
            np.concatenate([np.asarray(in_maps[c][nm]) for c in range(N_CORES)],
                           axis=0), sh)
        for nm in in_names
    ] + [
        jax.device_put(np.zeros((N_CORES * z.shape[0], *z.shape[1:]), z.dtype), sh)
        for z in zero_outs
    ]

    def run_once():
        outs = fn(*dev_args)
        jax.block_until_ready(outs)
        return [
            {nm: np.asarray(outs[i]).reshape(N_CORES, *out_avals[i].shape)[c]
             for i, nm in enumerate(out_names)}
            for c in range(N_CORES)
        ]

    def time_iters(n):
        ts = []
        for _ in range(n):
            t0 = time.perf_counter()
            jax.block_until_ready(fn(*dev_args))
            ts.append(time.perf_counter() - t0)
        return ts

    return run_once, time_iters


def kernel(q, k, v, mask, Wq, bq, Wk, bk, Wv, bv, Wo, bo):
    from concourse.bass_utils import run_bass_kernel_spmd

    q, k, v = (np.asarray(x) for x in (q, k, v))
    mask = np.asarray(mask)
    in_maps, nkt = _host_prep(q, k, v, mask, np.asarray(Wq), np.asarray(bq),
                              np.asarray(Wk), np.asarray(bk), np.asarray(Wv),
                              np.asarray(bv), np.asarray(Wo))
    key = (("bq" in in_maps[0]), ("bk" in in_maps[0]), ("bv" in in_maps[0]),
           nkt[0], nkt[1])
    if key not in _cached:
        _cached[key] = _build_program(*key)
    nc = _cached[key]

    trace = bool(int(os.environ.get("KERNEL_TRACE", "0")))
    res = run_bass_kernel_spmd(nc, in_maps, list(range(N_CORES)), trace=trace)
    kernel.last_results = res

    yT = np.zeros((D, T), np.float32)
    for i in range(N_CORES):
        yT += res.results[i]["y"].astype(np.float32)
    y = yT.T + np.asarray(bo).astype(np.float32)
    return np.ascontiguousarray(y.astype(np.float32)).reshape(B, L, D)


# revision 55
# speedup vs baseline: 1.1189x; 1.1189x over previous
"""Multi-head attention (B=2, L=2048, D=1024, H=16) on 8 Trainium2 NeuronCores.

Sharding: tensor-parallel over heads. Core c owns heads 2c, 2c+1, i.e. rows
[128c, 128c+128) of Wq/Wk/Wv and columns [128c, 128c+128) of Wo. Each core
computes Q/K/V projections for its 128 channels, attention for its 2 heads
(both batches), and a partial transposed out-projection yT_c = (attnO_c @
Wo[:, sl].T).T emitted in bf16. The host sums the 8 partials, transposes,
and adds bo.

Key optimizations over the naive layout:
- Key-padding-mask compaction: masked keys contribute exp(-inf)=0 exactly,
  so the host gathers only unmasked key/value tokens (padded to 128) and the
  kernel sizes its K/V projection + attention loops to the compacted length.
  With the ~50% random mask this halves scores/PV/exp work. Pad slots get
  bias -30000 so they exp to 0.0 like the reference's masked keys.
- All PE operands are bf16 (1 cycle/row; fp32 moving operands cost 4).
  Accumulation stays fp32 in PSUM.
- V is projected directly into [token, channel] layout (contraction chunks
  of x as the stationary operand), avoiding separate PE transposes.
- Scores are computed transposed (S.T tiles [k,q]) so softmax P.T lands in
  SBUF ready to be the PV matmul's moving operand; softmax max-subtraction
  is skipped (scores ~N(0,1), fp32 exp cannot overflow) and row sums ride
  along as a ones-column in the PV stationary, appearing as psum row 64.
- The out-projection is computed transposed (yT [D, T]): stationary
  woT chunks [128e, 128d] give contraction depth 128 (vs 64 the other way),
  halving out-proj PE time, and the bf16 yT output halves output DMA.
- Attention is software-pipelined with lag 2 (scores for kt issued two
  steps ahead of the PV accumulation of kt) so the PE never waits on the
  ACT exp; projection/output units are interleaved at generator yields.
"""

import os
import sys

for _p in ("/opt/trn_rl_repo", os.path.expanduser("~/.axon_site/_ro/trn_rl_repo")):
    if os.path.isdir(_p) and _p not in sys.path:
        sys.path.insert(0, _p)

import numpy as np

B = 2
L = 2048
D = 1024
T = B * L            # 4096 query tokens
E = 128              # channels per core (2 heads x 64)
HD = 64              # head dim
N_CORES = 8
SCALE = 1.0 / 8.0    # 1/sqrt(HD)
MASK_BIAS = -30000.0

N_DC = D // 128      # 8 contraction chunks
N_QT = L // 512      # 4 query tiles per batch

_cached = {}

# build-time experiment knobs (module-level; kernel() uses the defaults)
OPTS = {"split_kv": False, "halve_last": False, "v_mode": "direct",
        "y_mode": "singles_hwdge", "y_copy": "dve"}


def _build_program(has_bq, has_bk, has_bv, nkt0, nkt1, reps=1):
    import concourse.bacc as bacc
    import concourse.mybir as mybir
    import concourse.tile as tile

    F32 = mybir.dt.float32
    BF16 = mybir.dt.bfloat16
    AF = mybir.ActivationFunctionType
    ALU = mybir.AluOpType

    n_kt = (nkt0, nkt1)
    NG = nkt0 + nkt1               # 128-token key groups, both batches
    TK = 128 * NG                  # compacted+padded key tokens
    TKP = 512 * ((TK + 511) // 512)  # staged to 512-token proj tiles
    N_KVT = TKP // 512             # k/v projection token tiles
    N_QTT = T // 512               # q projection token tiles (8)
    goff = (0, nkt0)               # group offset per batch

    def mm(out, lhsT, rhs, **kw):
        nc.tensor.matmul(out, lhsT, rhs, **kw)

    nc = bacc.Bacc("TRN2", target_bir_lowering=False, debug=False,
                   num_devices=N_CORES)

    qT = nc.dram_tensor("qT", [D, T], BF16, kind="ExternalInput").ap()
    kT = nc.dram_tensor("kT", [D, TKP], BF16, kind="ExternalInput").ap()
    vT = nc.dram_tensor("vT", [D, TKP], BF16, kind="ExternalInput").ap()
    # w inputs are pre-chunked on the host: w[p, dc*128+e] = W.T[dc*128+p, e]
    # so the load is a plain contiguous DMA (2KB/partition descriptors).
    wq = nc.dram_tensor("wq", [128, D], BF16, kind="ExternalInput").ap()
    wk = nc.dram_tensor("wk", [128, D], BF16, kind="ExternalInput").ap()
    wv = nc.dram_tensor("wv", [128, D], BF16, kind="ExternalInput").ap()
    wo = nc.dram_tensor("wo", [E, D], BF16, kind="ExternalInput").ap()
    mbd = nc.dram_tensor("mb", [128, NG], F32, kind="ExternalInput").ap()
    onesd = nc.dram_tensor("ones128", [128, 128], BF16, kind="ExternalInput").ap()
    identd = nc.dram_tensor("ident", [128, 128], F32, kind="ExternalInput").ap() \
        if OPTS["v_mode"] == "transpose" else None
    bias_d = {}
    if has_bq:
        bias_d["q"] = nc.dram_tensor("bq", [128, 1], F32, kind="ExternalInput").ap()
    if has_bk:
        bias_d["k"] = nc.dram_tensor("bk", [128, 1], F32, kind="ExternalInput").ap()
    if has_bv:
        bias_d["v"] = nc.dram_tensor("bv", [64, 2], F32, kind="ExternalInput").ap()
    yd = nc.dram_tensor("y", [D, T], BF16, kind="ExternalOutput").ap()

    with tile.TileContext(nc) as tc:
        import contextlib
        with contextlib.ExitStack() as ctx:
            const = ctx.enter_context(tc.tile_pool(name="const", bufs=1))
            big = ctx.enter_context(tc.tile_pool(name="big", bufs=1))
            stg = ctx.enter_context(tc.tile_pool(name="stg", bufs=10))
            work = ctx.enter_context(tc.tile_pool(name="work", bufs=6))
            pt_pool = ctx.enter_context(tc.tile_pool(name="ptp", bufs=3))
            psum = ctx.enter_context(tc.tile_pool(name="psum", bufs=2, space="PSUM"))
            psst = ctx.enter_context(tc.tile_pool(name="psst", bufs=2, space="PSUM"))
            psacc = ctx.enter_context(tc.tile_pool(name="psacc", bufs=2, space="PSUM"))

            # ---- weights needed by the first projections (the remaining
            # constants are DMA'd inside rep 0 at scheduled points) ----
            # weight tiles (DMAs are emitted by the rep-0 startup sequence
            # below, in deadline order on a single queue)
            w_sb = {}
            w_src = {"k": wk, "v": wv, "q": wq}
            for nm in ("k", "v", "q"):
                w_sb[nm] = const.tile([128, D], BF16, name=f"w{nm}_sb")
            wo_sb = const.tile([128, D], BF16, name="wo_sb")
            mb_sb = const.tile([128, NG], F32, name="mb_sb")
            ones_sb = const.tile([128, 128], BF16, name="ones_sb")
            ident_sb = const.tile([128, 128], F32, name="ident_sb") \
                if OPTS["v_mode"] == "transpose" else None
            b_sb = {}
            for nm in bias_d:
                b_sb[nm] = const.tile([128, 2] if nm == "v" else [128, 1], F32,
                                      name=f"b{nm}_sb")

            # ---- persistent activations ----
            QT = big.tile([128, T], BF16, name="QT")     # [e, q-tok]
            KT = big.tile([128, TKP], BF16, name="KT")   # [e, k-tok]
            # V layout per 128-token group g: [V_headA(64) | 1 | V_headB(64) | 1]
            # partitions = k-token; the ones column rides along in the PV
            # matmul and produces the softmax denominator as psum row 64.
            V = big.tile([128, NG * 130], BF16, name="V")
            for g in range(NG):
                nc.gpsimd.memset(V[:, g * 130 + 64:g * 130 + 65], 1.0)
                nc.gpsimd.memset(V[:, g * 130 + 129:g * 130 + 130], 1.0)
            OT = big.tile([128, T], BF16, name="OT")     # [e(2 heads), q-tok]
            scr = big.tile([1, 8], F32, name="scr")      # ACT table preload out

            for rep in range(reps):
                srcs = {"q": qT, "k": kT, "v": vT}
                staged = {}

                # k/v token-column units: 512 wide, but cut at the first
                # 256-aligned boundary past batch 0 so batch-1's share of
                # that tile drops out of the startup-critical DMA chain;
                # the zero-padded tail is trimmed entirely.
                b0e = min(nkt0 * 128, TK) if OPTS["split_kv"] else TK
                cuts = list(range(0, b0e, 512))
                c = min(-(-b0e // 256) * 256, TK)
                if c > cuts[-1]:
                    cuts.append(c)
                while cuts[-1] < TK:
                    cuts.append(min(cuts[-1] + 512, TK))
                KVU = [(cuts[i], cuts[i + 1] - cuts[i])
                       for i in range(len(cuts) - 1)]
                N_KVU = len(KVU)
                n_b0u = sum(1 for c0, cw in KVU if c0 < b0e)

                def unit_cols(nm, u):
                    return (u * 512, 512) if nm == "q" else KVU[u]

                def proj_dma(nm, tt):
                    # one strided DMA lands all 8 contraction chunks
                    # side-by-side: s[p, dc*cw + t] = src[dc*128 + p, tt*512 + t]
                    # ALL input staging goes on the ONE scalar HWDGE queue:
                    # the DMA device serves transfer requests FIFO and HWDGE
                    # round-robins descriptor-gen between queues, so a
                    # single queue is the only way to control global
                    # transfer order (SWDGE gen is ~1us/DMA on Pool and
                    # loses the FIFO race entirely).
                    c0, cw = unit_cols(nm, tt)
                    s = stg.tile([128, N_DC * cw], BF16,
                                 name=f"{rep}_stg_{nm}_{tt}", tag="stg")
                    nc.scalar.dma_start(
                        s.rearrange("p (c t) -> p c t", c=N_DC),
                        srcs[nm][:, c0:c0 + cw]
                        .rearrange("(c p) t -> p c t", p=128))
                    staged[(nm, tt)] = s

                def proj_mm(nm, tt):
                    s = staged.pop((nm, tt))
                    c0, cw = unit_cols(nm, tt)
                    if nm in ("q", "k"):
                        ps = psum.tile([128, 512], F32,
                                       name=f"{rep}_ps_{nm}_{tt}", tag="mm")
                        for dc in range(N_DC):
                            mm(ps[:, 0:cw],
                               w_sb[nm][:, dc * 128:(dc + 1) * 128],
                               s[:, dc * cw:(dc + 1) * cw],
                               start=(dc == 0), stop=(dc == N_DC - 1))
                        dst = (QT if nm == "q" else KT)[:, c0:c0 + cw]
                        with nc.allow_low_precision(reason="bf16 activations"):
                            if nm in b_sb:
                                nc.vector.tensor_scalar(dst, ps[:, 0:cw],
                                                        b_sb[nm][:, 0:1],
                                                        None, ALU.add)
                            else:
                                nc.vector.tensor_copy(dst, ps[:, 0:cw])
                    elif OPTS["v_mode"] == "direct":
                        # V^T directly: out[t, e] accumulated over d-chunks
                        # with the x chunk as the stationary operand.
                        ps = psum.tile([128, 512], F32,
                                       name=f"{rep}_ps_v_{tt}", tag="mm")
                        for si in range(cw // 128):
                            if c0 // 128 + si >= NG:
                                break
                            for dc in range(N_DC):
                                mm(ps[:, si * 128:(si + 1) * 128],
                                   s[:, dc * cw + si * 128:dc * cw + (si + 1) * 128],
                                   w_sb["v"][:, dc * 128:(dc + 1) * 128],
                                   start=(dc == 0), stop=(dc == N_DC - 1))
                        with nc.allow_low_precision(reason="bf16 activations"):
                            for si in range(cw // 128):
                                g = c0 // 128 + si
                                if g >= NG:
                                    break
                                nc.vector.tensor_copy(
                                    V[:, g * 130:g * 130 + 64],
                                    ps[:, si * 128:si * 128 + 64])
                                nc.vector.tensor_copy(
                                    V[:, g * 130 + 65:g * 130 + 129],
                                    ps[:, si * 128 + 64:si * 128 + 128])
                    else:
                        # V like q/k ([e, tok]) then PE-transpose per group
                        ps = psum.tile([128, 512], F32,
                                       name=f"{rep}_ps_v_{tt}", tag="mm")
                        for dc in range(N_DC):
                            mm(ps[:, 0:cw],
                               w_sb["v"][:, dc * 128:(dc + 1) * 128],
                               s[:, dc * cw:(dc + 1) * cw],
                               start=(dc == 0), stop=(dc == N_DC - 1))
                        vs = work.tile([128, cw], F32,
                                       name=f"{rep}_vs_{tt}", tag="vs")
                        nc.vector.tensor_copy(vs[:], ps[:, 0:cw])
                        for si in range(cw // 128):
                            g = c0 // 128 + si
                            if g >= NG:
                                break
                            vtp = psum.tile([128, 128], F32,
                                            name=f"{rep}_vtp_{tt}_{si}",
                                            tag="mm")
                            nc.tensor.transpose(vtp[:],
                                                vs[:, si * 128:(si + 1) * 128],
                                                ident_sb[:])
                            with nc.allow_low_precision(reason="bf16"):
                                nc.vector.tensor_copy(
                                    V[:, g * 130:g * 130 + 64],
                                    vtp[:, 0:64])
                                nc.vector.tensor_copy(
                                    V[:, g * 130 + 65:g * 130 + 129],
                                    vtp[:, 64:128])

                # Software-pipelined attention for one (batch, 512-query
                # tile): scores for k-tile kt are issued on the PE two steps
                # before the PV accumulation of kt, so the PE never stalls on
                # the ACT exp of the tile it just produced. Yields carry
                # (pe_ns_just_emitted, key_groups_needed_next, at_normalize)
                # so the emitter can account PE time, force the K/V
                # projection units the next scores depend on, and slot
                # fillers into the dependency gaps.
                def attn_steps(b, qt, qo=0, qw=512):
                    q0 = b * L + qt * 512 + qo
                    nkt = n_kt[b]
                    kcost = (qw * 852) // 512
                    ot = [psacc.tile([65, qw], F32,
                                     name=f"{rep}_ot{h}_{b}_{qt}_{qo}",
                                     tag="acc")
                          for h in range(2)]

                    def scores(kt):
                        g = goff[b] + kt
                        k0 = g * 128
                        s = psst.tile([128, 2 * qw], F32,
                                      name=f"{rep}_st_{b}_{qt}_{qo}_{kt}",
                                      tag="st")
                        for h in range(2):
                            mm(s[:, h * qw:(h + 1) * qw],
                               KT[h * 64:(h + 1) * 64, k0:k0 + 128],
                               QT[h * 64:(h + 1) * 64, q0:q0 + qw],
                               start=True, stop=True)
                        p = pt_pool.tile([128, 2 * qw], BF16,
                                         name=f"{rep}_pt_{b}_{qt}_{qo}_{kt}",
                                         tag="pt")
                        nc.scalar.activation(p[:], s[:], AF.Exp,
                                             bias=mb_sb[:, g:g + 1],
                                             scale=SCALE)
                        return p

                    def pv(kt, p, last):
                        g = goff[b] + kt
                        for h in range(2):
                            mm(ot[h][:],
                               V[:, g * 130 + 65 * h: g * 130 + 65 * (h + 1)],
                               p[:, h * qw:(h + 1) * qw],
                               start=(kt == 0), stop=last)

                    gg = goff[b]
                    yield (0, [gg, gg + min(1, nkt - 1)], False)
                    pq = [scores(0)]
                    if nkt > 1:
                        pq.append(scores(1))
                    seg = kcost
                    for kt in range(2, nkt):
                        if kt % 2 == 0:
                            need = [gg + kt, gg + min(kt + 1, nkt - 1)]
                            yield (seg, need, False)
                            seg = 0
                        pq.append(scores(kt))
                        pv(kt - 2, pq.pop(0), last=False)
                        seg += kcost
                    for i, p in enumerate(pq):
                        kt = nkt - len(pq) + i
                        pv(kt, p, last=(kt == nkt - 1))
                        seg += kcost // 2

                    # normalize: OT rows = [otA/rA ; otB/rB]; 1/r broadcast
                    # across the 64 head partitions with a K=1 ones matmul
                    # (compute engines cannot move data across partitions).
                    # The yield lets PE fillers run while the DVE computes
                    # the reciprocals, instead of stalling on the bc matmul.
                    rb = [work.tile([65, qw], BF16,
                                    name=f"{rep}_rb{h}_{b}_{qt}_{qo}",
                                    tag=f"rb{h}")
                          for h in range(2)]
                    with nc.allow_low_precision(reason="feeds bf16 matmul"):
                        nc.vector.reciprocal(rb[0][64:65, :], ot[0][64:65, :])
                        nc.vector.reciprocal(rb[1][64:65, :], ot[1][64:65, :])
                    yield (seg, [], True)
                    bc_sb = work.tile([64, 2 * qw], F32,
                                      name=f"{rep}_bcs_{b}_{qt}_{qo}",
                                      tag="bcs")
                    for h in range(2):
                        bc_ps = psum.tile([64, qw], F32,
                                          name=f"{rep}_bc{h}_{b}_{qt}_{qo}",
                                          tag="mm")
                        mm(bc_ps[:], ones_sb[64:65, 0:64], rb[h][64:65, :],
                           start=True, stop=True)
                        nc.vector.tensor_copy(bc_sb[:, h * qw:(h + 1) * qw],
                                              bc_ps[:])
                    for h in range(2):
                        dst = OT[h * 64:(h + 1) * 64, q0:q0 + qw]
                        with nc.allow_low_precision(reason="bf16 attn output"):
                            nc.vector.tensor_mul(
                                dst, ot[h][0:64, :],
                                bc_sb[:, h * qw:(h + 1) * qw])
                        if "v" in b_sb:
                            nc.vector.tensor_scalar(dst, dst,
                                                    b_sb["v"][0:64, h:h + 1],
                                                    None, ALU.add)

                def y_unit(dc, c0, cw, tail=False):
                    # yT tiles [128 d, <=512 t]: contraction over all 128
                    # channels in one pass with the woT chunk stationary.
                    # Adjacent token tiles share one output DMA (via the
                    # otherwise-idle Pool engine's SWDGE) to halve the
                    # fixed per-DMA descriptor-generation cost.
                    ys = work.tile([128, cw], BF16,
                                   name=f"{rep}_ys_{dc}_{c0}", tag="ys")
                    for j in range(0, cw, 512):
                        w = min(512, cw - j)
                        # in the tail the scores pool is free: its extra
                        # PSUM banks deepen the mm->copy->DMA rotation
                        yp = (psst if tail else psum).tile(
                            [128, w], F32, name=f"{rep}_yp_{dc}_{c0 + j}",
                            tag="st" if tail else "mm")
                        mm(yp[:], wo_sb[:, dc * 128:(dc + 1) * 128],
                           OT[:, c0 + j:c0 + j + w],
                           start=True, stop=True)
                        # during attention the ACT engine is exp-bound: keep
                        # copies on the DVE; in the tail both are free.
                        with nc.allow_low_precision(reason="bf16 out"):
                            use_act = (tail or OPTS["y_copy"] == "alt") \
                                and (dc + j // 512) % 2 == 0
                            if use_act:
                                nc.scalar.copy(ys[:, j:j + w], yp[:])
                            else:
                                nc.vector.tensor_copy(ys[:, j:j + w], yp[:])
                    if tail or OPTS["y_mode"] == "singles_hwdge":
                        eng = nc.sync if dc % 2 else nc.scalar
                    else:
                        eng = nc.gpsimd
                    eng.dma_start(
                        yd[dc * 128:(dc + 1) * 128, c0:c0 + cw], ys[:])

                # ---- emission schedule ----
                # DMA transfers are serialized at HBM bandwidth, so the
                # issue order is the data arrival order; a static clock
                # estimate paces issues ~LEAD ns ahead of PE consumption
                # and gates optional filler matmuls on estimated arrival.
                STG_NS, LEAD = 2950.0, 9000.0
                est = {"dma": 0.0, "pe": 0.0}
                ready = {}

                def issue_dma(u):
                    proj_dma(*u)
                    _, cw = unit_cols(*u)
                    est["dma"] = max(est["dma"], est["pe"]) + (STG_NS * cw) / 512
                    ready[u] = est["dma"] + 500.0

                def mm_unit(u):
                    _, cw = unit_cols(*u)
                    est["pe"] = max(est["pe"], ready[u]) + (1707.0 * cw) / 512
                    proj_mm(*u)

                kv_all = [(nm, u) for u in range(N_KVU) for nm in ("k", "v")]
                pre = kv_all[:2 * n_b0u]
                rest = kv_all[2 * n_b0u:]
                # units in mm-emission order; DMA order (with the consts
                # woven in by deadline) is built separately below.
                dma_order = [("k", 0), ("v", 0), ("q", 0)] + pre[2:]
                dma_order += [("q", t) for t in range(1, min(5, N_QTT))]
                dma_order += rest
                dma_order += [("q", t) for t in range(5, N_QTT)]

                dma_pend = list(dma_order)
                mm_pend = list(dma_order)
                y_pend = []

                def covered_units(groups):
                    need = []
                    for g in groups:
                        x = g * 128
                        for u, (c0, cw) in enumerate(KVU):
                            if c0 <= x < c0 + cw:
                                for nm in ("k", "v"):
                                    un = (nm, u)
                                    if un in mm_pend and un not in need:
                                        need.append(un)
                    return need

                # startup DMA sequence, strictly in deadline order on the
                # single scalar queue: each weight/const lands just before
                # its first consumer, each stg tile as early as possible.
                np_pre = len(pre) + 1
                stg_pre, ci = dma_pend[:np_pre], 0

                def const_dma(dst, src):
                    if rep == 0:
                        nc.scalar.dma_start(dst[:], src[:])
                        est["dma"] += 200.0

                const_dma(w_sb["k"], w_src["k"])
                est["dma"] += 550.0
                issue_dma(stg_pre[0])                     # k0
                const_dma(w_sb["v"], w_src["v"])
                est["dma"] += 550.0
                issue_dma(stg_pre[1])                     # v0
                issue_dma(stg_pre[2])                     # q0
                const_dma(mb_sb, mbd)
                for nm, bt in b_sb.items():
                    const_dma(bt, bias_d[nm])
                const_dma(w_sb["q"], w_src["q"])
                est["dma"] += 550.0
                if ident_sb is not None:
                    const_dma(ident_sb, identd)
                if len(stg_pre) > 3:
                    issue_dma(stg_pre[3])                 # k1
                const_dma(ones_sb, onesd)
                for u in stg_pre[4:]:
                    issue_dma(u)                          # v1, k2, v2, ...
                del dma_pend[:np_pre]
                # load the ACT exp table during the idle startup window
                if rep == 0:
                    nc.scalar.activation(scr[0:1, 0:1], mb_sb[0:1, 0:1],
                                         AF.Exp, scale=1.0)
                for u in [("k", 0), ("v", 0), ("q", 0)]:
                    mm_unit(u)
                    mm_pend.remove(u)
                const_dma(wo_sb, wo)

                tiles = [(b, qt) for b in range(B) for qt in range(N_QT)]
                last_ti = len(tiles) - 1
                # the last tile runs as two half-width query windows so its
                # normalize + output drain overlaps the second half's
                # attention instead of sitting entirely in the tail.
                HALVE_LAST = OPTS["halve_last"]
                subtiles = []
                for ti in range(len(tiles)):
                    b, qt = tiles[ti]
                    if ti == last_ti and HALVE_LAST:
                        subtiles.append((ti, b, qt, 0, 256))
                        subtiles.append((ti, b, qt, 256, 256))
                    else:
                        subtiles.append((ti, b, qt, 0, 512))

                def run_subtile(ti, b, qt, qo, qw, final):
                    for cost, need, at_norm in attn_steps(b, qt, qo, qw):
                        est["pe"] += cost
                        while dma_pend and est["dma"] < est["pe"] + LEAD:
                            issue_dma(dma_pend.pop(0))
                        budget = 1800.0
                        # forced: K/V units the next scores depend on, and
                        # the next tile's q projection by its deadline.
                        forced = covered_units(need)
                        if at_norm and ("q", ti + 1) in mm_pend:
                            forced.append(("q", ti + 1))
                        for u in forced:
                            while u in dma_pend:  # must be issued by now
                                issue_dma(dma_pend.pop(0))
                            mm_unit(u)
                            mm_pend.remove(u)
                            budget -= 1707
                        if at_norm and final:
                            # final normalize: nothing else will fill the
                            # PE while the DVE reciprocals run - drain all
                            # ready y units here (copies on ACT: exp done)
                            for dc, c0, cw in y_pend:
                                y_unit(dc, c0, cw, tail=True)
                            del y_pend[:]
                            continue
                        # optional fillers: proj mms whose data has landed,
                        # then ready y units (max 2: deeper bursts stall on
                        # the 2-buffer PSUM pool rotation).
                        y_n = 0
                        while budget > 0:
                            pick = None
                            for u in mm_pend[:2]:
                                if (u in ready
                                        and ready[u] <= est["pe"] + 400
                                        and budget >= 1707):
                                    pick = u
                                    break
                            if pick is not None:
                                mm_unit(pick)
                                mm_pend.remove(pick)
                                budget -= 1707
                            elif (y_pend
                                    and y_n < 2
                                    and budget >= (250 * y_pend[0][2]) // 512):
                                dc, c0, cw = y_pend.pop(0)
                                y_unit(dc, c0, cw)
                                cost_y = (250 * cw) // 512
                                est["pe"] += cost_y
                                budget -= cost_y
                                y_n += 1
                            else:
                                break

                for si, (ti, b, qt, qo, qw) in enumerate(subtiles):
                    if qo == 0:
                        assert ("q", ti) not in mm_pend, f"q{ti} not emitted"
                    run_subtile(ti, b, qt, qo, qw,
                                final=(si == len(subtiles) - 1))
                    # this query window is normalized -> its y units are
                    # ready (full tiles pair up across odd ti for one DMA)
                    c0 = ti * 512 + qo
                    pair = OPTS["y_mode"] == "pairs_swdge"
                    if qw == 512 and pair:
                        if ti % 2 == 1 and ti < 6:
                            y_pend.extend((dc, c0 - 512, 1024)
                                          for dc in range(N_DC))
                        elif ti >= 6:
                            y_pend.extend((dc, c0, 512)
                                          for dc in range(N_DC))
                    else:
                        y_pend.extend((dc, c0, qw) for dc in range(N_DC))
                for u in list(mm_pend):
                    while u in dma_pend:
                        issue_dma(dma_pend.pop(0))
                    mm_unit(u)
                    mm_pend.remove(u)
                for dc, c0, cw in y_pend:
                    y_unit(dc, c0, cw, tail=True)

    nc.compile()
    return nc


def _host_prep(q, k, v, mask, Wq, bq, Wk, bk, Wv, bv, Wo):
    """Build the per-core input maps. Compacts masked keys out of k/v."""
    import ml_dtypes
    f32 = np.float32
    bf16 = ml_dtypes.bfloat16

    qT = np.ascontiguousarray(q.reshape(T, D).T.astype(bf16))

    # --- key compaction: keep only unmasked tokens, pad groups to 128 ---
    idxs, biases, nkts = [], [], []
    for b in range(B):
        idx = np.flatnonzero(~mask[b])
        nkt = max(1, (len(idx) + 127) // 128)
        pad = 128 * nkt - len(idx)
        bias = np.concatenate([np.zeros(len(idx), f32),
                               np.full(pad, MASK_BIAS, f32)])
        idx = np.concatenate([idx, np.zeros(pad, np.int64)])
        idxs.append(idx)
        biases.append(bias)
        nkts.append(nkt)
    NG = sum(nkts)
    TK = 128 * NG
    TKP = 512 * ((TK + 511) // 512)
    kc = np.concatenate([k[b][idxs[b]] for b in range(B)], axis=0)
    vc = np.concatenate([v[b][idxs[b]] for b in range(B)], axis=0)
    kc = np.concatenate([kc, np.zeros((TKP - TK, D), kc.dtype)], axis=0)
    vc = np.concatenate([vc, np.zeros((TKP - TK, D), vc.dtype)], axis=0)
    kT = np.ascontiguousarray(kc.T.astype(bf16))
    vT = np.ascontiguousarray(vc.T.astype(bf16))
    mb = np.concatenate(biases).reshape(NG, 128).T
    mb = np.ascontiguousarray(mb.astype(f32))
    ones128 = np.ones((128, 128), bf16)
    ident = np.eye(128, dtype=f32)

    def chunked(wT):
        # [D, E] -> [128, N_DC*E]: w[p, dc*E + e] = wT[dc*128 + p, e]
        return np.ascontiguousarray(
            wT.reshape(N_DC, 128, E).transpose(1, 0, 2).reshape(128, D))

    in_maps = []
    for c in range(N_CORES):
        sl = slice(c * E, (c + 1) * E)
        m = {
            "qT": qT, "kT": kT, "vT": vT,
            "wq": chunked(Wq[sl, :].T.astype(bf16)),
            "wk": chunked(Wk[sl, :].T.astype(bf16)),
            "wv": chunked(Wv[sl, :].T.astype(bf16)),
            "wo": np.ascontiguousarray(Wo[:, sl].T.astype(bf16)),
            "mb": mb, "ones128": ones128, "ident": ident,
        }
        if np.any(bq):
            m["bq"] = np.ascontiguousarray(bq[sl].astype(f32).reshape(128, 1))
        if np.any(bk):
            m["bk"] = np.ascontiguousarray(bk[sl].astype(f32).reshape(128, 1))
        if np.any(bv):
            m["bv"] = np.ascontiguousarray(bv[sl].astype(f32).reshape(2, 64).T)
        in_maps.append(m)
    return in_maps, (nkts[0], nkts[1])


def _make_timed_runner(nc, in_maps):
    """Build a reusable jitted runner for `nc` (no output donation — the
    program writes every output element, so uninit result buffers are fine).
    Returns (run_once() -> per-core outputs as numpy, time_iters(n) -> [sec])."""
    import jax
    import time
    import concourse.mybir as mybir
    from concourse import bass2jax
    from jax.experimental.shard_map import shard_map
    from jax.sharding import Mesh, NamedSharding, PartitionSpec

    bass2jax.install_neuronx_cc_hook()

    partition_name = nc.partition_id_tensor.name if nc.partition_id_tensor else None
    in_names, out_names, out_avals, zero_outs = [], [], [], []
    for alloc in nc.m.functions[0].allocations:
        if not isinstance(alloc, mybir.MemoryLocationSet):
            continue
        name = alloc.memorylocations[0].name
        if alloc.kind == "ExternalInput":
            if name != partition_name:
                in_names.append(name)
        elif alloc.kind == "ExternalOutput":
            shape = tuple(alloc.tensor_shape)
            dtype = mybir.dt.np(alloc.dtype)
            out_names.append(name)
            out_avals.append(jax.core.ShapedArray(shape, dtype))
            zero_outs.append(np.zeros(shape, dtype))
    n_params = len(in_names)
    all_in_names = list(in_names) + list(out_names)
    if partition_name is not None:
        all_in_names.append(partition_name)

    def _body(*args):
        operands = list(args)
        if partition_name is not None:
            operands.append(bass2jax.partition_id_tensor())
        outs = bass2jax._bass_exec_p.bind(
            *operands,
            out_avals=tuple(out_avals),
            in_names=tuple(all_in_names),
            out_names=tuple(out_names),
            lowering_input_output_aliases=(),
            sim_require_finite=True,
            sim_require_nnan=True,
            nc=nc,
        )
        return tuple(outs)

    devices = jax.devices()[:N_CORES]
    mesh = Mesh(np.asarray(devices), ("core",))
    nin = n_params + len(out_names)
    fn = jax.jit(shard_map(_body, mesh=mesh,
                           in_specs=(PartitionSpec("core"),) * nin,
                           out_specs=(PartitionSpec("core"),) * len(out_names),
                           check_rep=False))
    sh = NamedSharding(mesh, PartitionSpec("core"))
    dev_args = [
        jax.device_put(
            np.concatenate([np.asarray(in_maps[c][nm]) for c in range(N_CORES)],
                           axis=0), sh)
        for nm in in_names
    ] + [
        jax.device_put(np.zeros((N_CORES * z.shape[0], *z.shape[1:]), z.dtype), sh)
        for z in zero_outs
    ]

    def run_once():
        outs = fn(*dev_args)
        jax.block_until_ready(outs)
        return [
            {nm: np.asarray(outs[i]).reshape(N_CORES, *out_avals[i].shape)[c]
             for i, nm in enumerate(out_names)}
            for c in range(N_CORES)
        ]

    def time_iters(n):
        ts = []
        for _ in range(n):
            t0 = time.perf_counter()
            jax.block_until_ready(fn(*dev_args))
            ts.append(time.perf_counter() - t0)
        return ts

    return run_once, time_iters


def kernel(q, k, v, mask, Wq, bq, Wk, bk, Wv, bv, Wo, bo):
    from concourse.bass_utils import run_bass_kernel_spmd

    q, k, v = (np.asarray(x) for x in (q, k, v))
    mask = np.asarray(mask)
    in_maps, nkt = _host_prep(q, k, v, mask, np.asarray(Wq), np.asarray(bq),
                              np.asarray(Wk), np.asarray(bk), np.asarray(Wv),
                              np.asarray(bv), np.asarray(Wo))
    key = (("bq" in in_maps[0]), ("bk" in in_maps[0]), ("bv" in in_maps[0]),
           nkt[0], nkt[1])
    if key not in _cached:
        _cached[key] = _build_program(*key)
    nc = _cached[key]

    trace = bool(int(os.environ.get("KERNEL_TRACE", "0")))
    res = run_bass_kernel_spmd(nc, in_maps, list(range(N_CORES)), trace=trace)
    kernel.last_results = res

    yT = np.zeros((D, T), np.float32)
    for i in range(N_CORES):
        yT += res.results[i]["y"].astype(np.float32)
    y = yT.T + np.asarray(bo).astype(np.float32)
    return np.ascontiguousarray(y.astype(np.float32)).reshape(B, L, D)


# revision 62
# speedup vs baseline: 1.1682x; 1.0441x over previous
"""Multi-head attention (B=2, L=2048, D=1024, H=16) on 8 Trainium2 NeuronCores.

Sharding: tensor-parallel over heads. Core c owns heads 2c, 2c+1, i.e. rows
[128c, 128c+128) of Wq/Wk/Wv and columns [128c, 128c+128) of Wo. Each core
computes Q/K/V projections for its 128 channels, attention for its 2 heads
(both batches), and a partial transposed out-projection yT_c = (attnO_c @
Wo[:, sl].T).T emitted in bf16. The host sums the 8 partials, transposes,
and adds bo.

Key optimizations over the naive layout:
- Key-padding-mask compaction: masked keys contribute exp(-inf)=0 exactly,
  so the host gathers only unmasked key/value tokens (padded to 128) and the
  kernel sizes its K/V projection + attention loops to the compacted length.
  With the ~50% random mask this halves scores/PV/exp work. Pad slots get
  bias -30000 so they exp to 0.0 like the reference's masked keys.
- All PE operands are bf16 (1 cycle/row; fp32 moving operands cost 4).
  Accumulation stays fp32 in PSUM.
- V is projected directly into [token, channel] layout (contraction chunks
  of x as the stationary operand), avoiding separate PE transposes.
- Scores are computed transposed (S.T tiles [k,q]) so softmax P.T lands in
  SBUF ready to be the PV matmul's moving operand; softmax max-subtraction
  is skipped (scores ~N(0,1), fp32 exp cannot overflow) and row sums ride
  along as a ones-column in the PV stationary, appearing as psum row 64.
- The out-projection is computed transposed (yT [D, T]): stationary
  woT chunks [128e, 128d] give contraction depth 128 (vs 64 the other way),
  halving out-proj PE time, and the bf16 yT output halves output DMA.
- Attention is software-pipelined with lag 2 (scores for kt issued two
  steps ahead of the PV accumulation of kt) so the PE never waits on the
  ACT exp; projection/output units are interleaved at generator yields.
"""

import os
import sys

for _p in ("/opt/trn_rl_repo", os.path.expanduser("~/.axon_site/_ro/trn_rl_repo")):
    if os.path.isdir(_p) and _p not in sys.path:
        sys.path.insert(0, _p)

import numpy as np

B = 2
L = 2048
D = 1024
T = B * L            # 4096 query tokens
E = 128              # channels per core (2 heads x 64)
HD = 64              # head dim
N_CORES = 8
SCALE = 1.0 / 8.0    # 1/sqrt(HD)
MASK_BIAS = -30000.0

N_DC = D // 128      # 8 contraction chunks
N_QT = L // 512      # 4 query tiles per batch

_cached = {}

# build-time experiment knobs (module-level; kernel() uses the defaults)
OPTS = {"split_kv": False, "halve_last": False, "v_mode": "direct",
        "y_mode": "singles_hwdge", "y_copy": "dve", "scores": "merged",
        "bc_direct": False}


def _build_program(has_bq, has_bk, has_bv, nkt0, nkt1, reps=1):
    import concourse.bacc as bacc
    import concourse.mybir as mybir
    import concourse.tile as tile

    F32 = mybir.dt.float32
    BF16 = mybir.dt.bfloat16
    AF = mybir.ActivationFunctionType
    ALU = mybir.AluOpType

    n_kt = (nkt0, nkt1)
    NG = nkt0 + nkt1               # 128-token key groups, both batches
    TK = 128 * NG                  # compacted+padded key tokens
    TKP = 512 * ((TK + 511) // 512)  # staged to 512-token proj tiles
    N_KVT = TKP // 512             # k/v projection token tiles
    N_QTT = T // 512               # q projection token tiles (8)
    goff = (0, nkt0)               # group offset per batch

    def mm(out, lhsT, rhs, **kw):
        nc.tensor.matmul(out, lhsT, rhs, **kw)

    nc = bacc.Bacc("TRN2", target_bir_lowering=False, debug=False,
                   num_devices=N_CORES)

    qT = nc.dram_tensor("qT", [D, T], BF16, kind="ExternalInput").ap()
    kT = nc.dram_tensor("kT", [D, TKP], BF16, kind="ExternalInput").ap()
    vT = nc.dram_tensor("vT", [D, TKP], BF16, kind="ExternalInput").ap()
    # w inputs are pre-chunked on the host: w[p, dc*128+e] = W.T[dc*128+p, e]
    # so the load is a plain contiguous DMA (2KB/partition descriptors).
    wq = nc.dram_tensor("wq", [128, D], BF16, kind="ExternalInput").ap()
    wk = nc.dram_tensor("wk", [128, D], BF16, kind="ExternalInput").ap()
    wv = nc.dram_tensor("wv", [128, D], BF16, kind="ExternalInput").ap()
    wo = nc.dram_tensor("wo", [E, D], BF16, kind="ExternalInput").ap()
    mbd = nc.dram_tensor("mb", [128, NG], F32, kind="ExternalInput").ap()
    onesd = nc.dram_tensor("ones128", [128, 128], BF16, kind="ExternalInput").ap()
    identd = nc.dram_tensor("ident", [128, 128], F32, kind="ExternalInput").ap() \
        if OPTS["v_mode"] == "transpose" else None
    bias_d = {}
    if has_bq:
        bias_d["q"] = nc.dram_tensor("bq", [128, 1], F32, kind="ExternalInput").ap()
    if has_bk:
        bias_d["k"] = nc.dram_tensor("bk", [128, 1], F32, kind="ExternalInput").ap()
    if has_bv:
        bias_d["v"] = nc.dram_tensor("bv", [64, 2], F32, kind="ExternalInput").ap()
    yd = nc.dram_tensor("y", [D, T], BF16, kind="ExternalOutput").ap()

    with tile.TileContext(nc) as tc:
        import contextlib
        with contextlib.ExitStack() as ctx:
            const = ctx.enter_context(tc.tile_pool(name="const", bufs=1))
            big = ctx.enter_context(tc.tile_pool(name="big", bufs=1))
            stg = ctx.enter_context(tc.tile_pool(name="stg", bufs=10))
            work = ctx.enter_context(tc.tile_pool(name="work", bufs=6))
            pt_pool = ctx.enter_context(tc.tile_pool(name="ptp", bufs=3))
            psum = ctx.enter_context(tc.tile_pool(name="psum", bufs=2, space="PSUM"))
            psst = ctx.enter_context(tc.tile_pool(name="psst", bufs=2, space="PSUM"))
            psacc = ctx.enter_context(tc.tile_pool(name="psacc", bufs=2, space="PSUM"))

            # ---- weights needed by the first projections (the remaining
            # constants are DMA'd inside rep 0 at scheduled points) ----
            # weight tiles (DMAs are emitted by the rep-0 startup sequence
            # below, in deadline order on a single queue)
            w_sb = {}
            w_src = {"k": wk, "v": wv, "q": wq}
            for nm in ("k", "v", "q"):
                w_sb[nm] = const.tile([128, D], BF16, name=f"w{nm}_sb")
            wo_sb = const.tile([128, D], BF16, name="wo_sb")
            mb_sb = const.tile([128, NG], F32, name="mb_sb")
            ones_sb = const.tile([128, 128], BF16, name="ones_sb")
            ident_sb = const.tile([128, 128], F32, name="ident_sb") \
                if OPTS["v_mode"] == "transpose" else None
            b_sb = {}
            for nm in bias_d:
                b_sb[nm] = const.tile([128, 2] if nm == "v" else [128, 1], F32,
                                      name=f"b{nm}_sb")

            # ---- persistent activations ----
            MERGED = OPTS["scores"] == "merged"
            if MERGED:
                # stacked-zero Q, interleaved per 512-token tile: block 2i =
                # [QA_i; 0], block 2i+1 = [0; QB_i]. One 128-deep matmul on a
                # contiguous [128, 1024] window then computes both heads'
                # scores against the head-stacked KT (the zero blocks kill
                # the cross-head terms).
                QT = big.tile([128, 2 * T], BF16, name="QS")
                for i in range(T // 512):
                    nc.gpsimd.memset(QT[64:128, i * 1024:i * 1024 + 512], 0.0)
                    nc.gpsimd.memset(QT[0:64, i * 1024 + 512:(i + 1) * 1024], 0.0)
            else:
                QT = big.tile([128, T], BF16, name="QT")     # [e, q-tok]
            KT = big.tile([128, TKP], BF16, name="KT")   # [e, k-tok]
            # V layout per 128-token group g: [V_headA(64) | 1 | V_headB(64) | 1]
            # partitions = k-token; the ones column rides along in the PV
            # matmul and produces the softmax denominator as psum row 64.
            V = big.tile([128, NG * 130], BF16, name="V")
            for g in range(NG):
                nc.gpsimd.memset(V[:, g * 130 + 64:g * 130 + 65], 1.0)
                nc.gpsimd.memset(V[:, g * 130 + 129:g * 130 + 130], 1.0)
            OT = big.tile([128, T], BF16, name="OT")     # [e(2 heads), q-tok]
            scr = big.tile([1, 8], F32, name="scr")      # ACT table preload out

            for rep in range(reps):
                srcs = {"q": qT, "k": kT, "v": vT}
                staged = {}

                # k/v token-column units: 512 wide, but cut at the first
                # 256-aligned boundary past batch 0 so batch-1's share of
                # that tile drops out of the startup-critical DMA chain;
                # the zero-padded tail is trimmed entirely.
                b0e = min(nkt0 * 128, TK) if OPTS["split_kv"] else TK
                cuts = list(range(0, b0e, 512))
                c = min(-(-b0e // 256) * 256, TK)
                if c > cuts[-1]:
                    cuts.append(c)
                while cuts[-1] < TK:
                    cuts.append(min(cuts[-1] + 512, TK))
                KVU = [(cuts[i], cuts[i + 1] - cuts[i])
                       for i in range(len(cuts) - 1)]
                N_KVU = len(KVU)
                n_b0u = sum(1 for c0, cw in KVU if c0 < b0e)

                def unit_cols(nm, u):
                    return (u * 512, 512) if nm == "q" else KVU[u]

                def proj_dma(nm, tt):
                    # one strided DMA lands all 8 contraction chunks
                    # side-by-side: s[p, dc*cw + t] = src[dc*128 + p, tt*512 + t]
                    # ALL input staging goes on the ONE scalar HWDGE queue:
                    # the DMA device serves transfer requests FIFO and HWDGE
                    # round-robins descriptor-gen between queues, so a
                    # single queue is the only way to control global
                    # transfer order (SWDGE gen is ~1us/DMA on Pool and
                    # loses the FIFO race entirely).
                    c0, cw = unit_cols(nm, tt)
                    s = stg.tile([128, N_DC * cw], BF16,
                                 name=f"{rep}_stg_{nm}_{tt}", tag="stg")
                    nc.scalar.dma_start(
                        s.rearrange("p (c t) -> p c t", c=N_DC),
                        srcs[nm][:, c0:c0 + cw]
                        .rearrange("(c p) t -> p c t", p=128))
                    staged[(nm, tt)] = s

                def proj_mm(nm, tt):
                    s = staged.pop((nm, tt))
                    c0, cw = unit_cols(nm, tt)
                    if nm in ("q", "k"):
                        ps = psum.tile([128, 512], F32,
                                       name=f"{rep}_ps_{nm}_{tt}", tag="mm")
                        for dc in range(N_DC):
                            mm(ps[:, 0:cw],
                               w_sb[nm][:, dc * 128:(dc + 1) * 128],
                               s[:, dc * cw:(dc + 1) * cw],
                               start=(dc == 0), stop=(dc == N_DC - 1))
                        if nm == "q" and MERGED:
                            dsts = [(QT[0:64, 2 * c0:2 * c0 + cw],
                                     ps[0:64, 0:cw], slice(0, 64)),
                                    (QT[64:128, 2 * c0 + 512:2 * c0 + 512 + cw],
                                     ps[64:128, 0:cw], slice(64, 128))]
                        else:
                            dsts = [((QT if nm == "q" else KT)[:, c0:c0 + cw],
                                     ps[:, 0:cw], slice(0, 128))]
                        with nc.allow_low_precision(reason="bf16 activations"):
                            for dst, src, prt in dsts:
                                if nm in b_sb:
                                    nc.vector.tensor_scalar(
                                        dst, src, b_sb[nm][prt, 0:1],
                                        None, ALU.add)
                                else:
                                    nc.vector.tensor_copy(dst, src)
                    elif OPTS["v_mode"] == "direct":
                        # V^T directly: out[t, e] accumulated over d-chunks
                        # with the x chunk as the stationary operand.
                        ps = psum.tile([128, 512], F32,
                                       name=f"{rep}_ps_v_{tt}", tag="mm")
                        for si in range(cw // 128):
                            if c0 // 128 + si >= NG:
                                break
                            for dc in range(N_DC):
                                mm(ps[:, si * 128:(si + 1) * 128],
                                   s[:, dc * cw + si * 128:dc * cw + (si + 1) * 128],
                                   w_sb["v"][:, dc * 128:(dc + 1) * 128],
                                   start=(dc == 0), stop=(dc == N_DC - 1))
                        with nc.allow_low_precision(reason="bf16 activations"):
                            for si in range(cw // 128):
                                g = c0 // 128 + si
                                if g >= NG:
                                    break
                                nc.vector.tensor_copy(
                                    V[:, g * 130:g * 130 + 64],
                                    ps[:, si * 128:si * 128 + 64])
                                nc.vector.tensor_copy(
                                    V[:, g * 130 + 65:g * 130 + 129],
                                    ps[:, si * 128 + 64:si * 128 + 128])
                    else:
                        # V like q/k ([e, tok]) then PE-transpose per group
                        ps = psum.tile([128, 512], F32,
                                       name=f"{rep}_ps_v_{tt}", tag="mm")
                        for dc in range(N_DC):
                            mm(ps[:, 0:cw],
                               w_sb["v"][:, dc * 128:(dc + 1) * 128],
                               s[:, dc * cw:(dc + 1) * cw],
                               start=(dc == 0), stop=(dc == N_DC - 1))
                        vs = work.tile([128, cw], F32,
                                       name=f"{rep}_vs_{tt}", tag="vs")
                        nc.vector.tensor_copy(vs[:], ps[:, 0:cw])
                        for si in range(cw // 128):
                            g = c0 // 128 + si
                            if g >= NG:
                                break
                            vtp = psum.tile([128, 128], F32,
                                            name=f"{rep}_vtp_{tt}_{si}",
                                            tag="mm")
                            nc.tensor.transpose(vtp[:],
                                                vs[:, si * 128:(si + 1) * 128],
                                                ident_sb[:])
                            with nc.allow_low_precision(reason="bf16"):
                                nc.vector.tensor_copy(
                                    V[:, g * 130:g * 130 + 64],
                                    vtp[:, 0:64])
                                nc.vector.tensor_copy(
                                    V[:, g * 130 + 65:g * 130 + 129],
                                    vtp[:, 64:128])

                # Software-pipelined attention for one (batch, 512-query
                # tile): scores for k-tile kt are issued on the PE two steps
                # before the PV accumulation of kt, so the PE never stalls on
                # the ACT exp of the tile it just produced. Yields carry
                # (pe_ns_just_emitted, key_groups_needed_next, at_normalize)
                # so the emitter can account PE time, force the K/V
                # projection units the next scores depend on, and slot
                # fillers into the dependency gaps.
                def attn_steps(b, qt, qo=0, qw=512):
                    assert qw == 512 or not MERGED, "merged scores need qw=512"
                    q0 = b * L + qt * 512 + qo
                    nkt = n_kt[b]
                    kcost = (qw * 852) // 512
                    ot = [psacc.tile([65, qw], F32,
                                     name=f"{rep}_ot{h}_{b}_{qt}_{qo}",
                                     tag="acc")
                          for h in range(2)]

                    def scores(kt):
                        g = goff[b] + kt
                        k0 = g * 128
                        s = psst.tile([128, 2 * qw], F32,
                                      name=f"{rep}_st_{b}_{qt}_{qo}_{kt}",
                                      tag="st")
                        if MERGED:
                            # two 512-col mms (PSUM-bank ISA limit) sharing
                            # one 128-deep stationary: no per-head reload
                            for h in range(2):
                                mm(s[:, h * qw:(h + 1) * qw],
                                   KT[:, k0:k0 + 128],
                                   QT[:, 2 * q0 + h * qw:2 * q0 + (h + 1) * qw],
                                   start=True, stop=True)
                        else:
                            for h in range(2):
                                mm(s[:, h * qw:(h + 1) * qw],
                                   KT[h * 64:(h + 1) * 64, k0:k0 + 128],
                                   QT[h * 64:(h + 1) * 64, q0:q0 + qw],
                                   start=True, stop=True)
                        p = pt_pool.tile([128, 2 * qw], BF16,
                                         name=f"{rep}_pt_{b}_{qt}_{qo}_{kt}",
                                         tag="pt")
                        nc.scalar.activation(p[:], s[:], AF.Exp,
                                             bias=mb_sb[:, g:g + 1],
                                             scale=SCALE)
                        return p

                    def pv(kt, p, last):
                        g = goff[b] + kt
                        for h in range(2):
                            mm(ot[h][:],
                               V[:, g * 130 + 65 * h: g * 130 + 65 * (h + 1)],
                               p[:, h * qw:(h + 1) * qw],
                               start=(kt == 0), stop=last)

                    gg = goff[b]
                    yield (0, [gg, gg + min(1, nkt - 1)], False)
                    pq = [scores(0)]
                    if nkt > 1:
                        pq.append(scores(1))
                    seg = kcost
                    for kt in range(2, nkt):
                        if kt % 2 == 0:
                            need = [gg + kt, gg + min(kt + 1, nkt - 1)]
                            yield (seg, need, False)
                            seg = 0
                        pq.append(scores(kt))
                        pv(kt - 2, pq.pop(0), last=False)
                        seg += kcost
                    for i, p in enumerate(pq):
                        kt = nkt - len(pq) + i
                        pv(kt, p, last=(kt == nkt - 1))
                        seg += kcost // 2

                    # normalize: OT rows = [otA/rA ; otB/rB]; 1/r broadcast
                    # across the 64 head partitions with a K=1 ones matmul
                    # (compute engines cannot move data across partitions).
                    # The yield lets PE fillers run while the DVE computes
                    # the reciprocals, instead of stalling on the bc matmul.
                    rb = [work.tile([65, qw], BF16,
                                    name=f"{rep}_rb{h}_{b}_{qt}_{qo}",
                                    tag=f"rb{h}")
                          for h in range(2)]
                    with nc.allow_low_precision(reason="feeds bf16 matmul"):
                        nc.vector.reciprocal(rb[0][64:65, :], ot[0][64:65, :])
                        nc.vector.reciprocal(rb[1][64:65, :], ot[1][64:65, :])
                    yield (seg, [], True)
                    bc_p = []
                    bc_sb = None if OPTS["bc_direct"] else work.tile(
                        [64, 2 * qw], F32, name=f"{rep}_bcs_{b}_{qt}_{qo}",
                        tag="bcs")
                    for h in range(2):
                        bc_ps = psum.tile([64, qw], F32,
                                          name=f"{rep}_bc{h}_{b}_{qt}_{qo}",
                                          tag="mm")
                        mm(bc_ps[:], ones_sb[64:65, 0:64], rb[h][64:65, :],
                           start=True, stop=True)
                        if bc_sb is None:
                            bc_p.append(bc_ps)
                        else:
                            nc.vector.tensor_copy(
                                bc_sb[:, h * qw:(h + 1) * qw], bc_ps[:])
                    for h in range(2):
                        dst = OT[h * 64:(h + 1) * 64, q0:q0 + qw]
                        src = bc_p[h][:] if bc_sb is None \
                            else bc_sb[:, h * qw:(h + 1) * qw]
                        with nc.allow_low_precision(reason="bf16 attn output"):
                            nc.vector.tensor_mul(dst, ot[h][0:64, :], src)
                        if "v" in b_sb:
                            nc.vector.tensor_scalar(dst, dst,
                                                    b_sb["v"][0:64, h:h + 1],
                                                    None, ALU.add)

                def y_unit(dc, c0, cw, tail=False):
                    # yT tiles [128 d, <=512 t]: contraction over all 128
                    # channels in one pass with the woT chunk stationary.
                    # Output DMAs ride the two HWDGE queues (measured faster
                    # than batching pairs through the Pool engine's SWDGE).
                    ys = work.tile([128, cw], BF16,
                                   name=f"{rep}_ys_{dc}_{c0}", tag="ys")
                    for j in range(0, cw, 512):
                        w = min(512, cw - j)
                        # in the tail the scores pool is free: its extra
                        # PSUM banks deepen the mm->copy->DMA rotation
                        yp = (psst if tail else psum).tile(
                            [128, w], F32, name=f"{rep}_yp_{dc}_{c0 + j}",
                            tag="st" if tail else "mm")
                        mm(yp[:], wo_sb[:, dc * 128:(dc + 1) * 128],
                           OT[:, c0 + j:c0 + j + w],
                           start=True, stop=True)
                        # during attention the ACT engine is exp-bound: keep
                        # copies on the DVE; in the tail both are free.
                        with nc.allow_low_precision(reason="bf16 out"):
                            use_act = (tail or OPTS["y_copy"] == "alt") \
                                and (dc + j // 512) % 2 == 0
                            if use_act:
                                nc.scalar.copy(ys[:, j:j + w], yp[:])
                            else:
                                nc.vector.tensor_copy(ys[:, j:j + w], yp[:])
                    if tail or OPTS["y_mode"] != "pairs_swdge":
                        eng = nc.sync if dc % 2 else nc.scalar
                    else:
                        eng = nc.gpsimd
                    eng.dma_start(
                        yd[dc * 128:(dc + 1) * 128, c0:c0 + cw], ys[:])

                # ---- emission schedule ----
                # DMA transfers are serialized at HBM bandwidth, so the
                # issue order is the data arrival order; a static clock
                # estimate paces issues ~LEAD ns ahead of PE consumption
                # and gates optional filler matmuls on estimated arrival.
                STG_NS, LEAD = 2950.0, 9000.0
                est = {"dma": 0.0, "pe": 0.0}
                ready = {}

                def issue_dma(u):
                    proj_dma(*u)
                    _, cw = unit_cols(*u)
                    est["dma"] = max(est["dma"], est["pe"]) + (STG_NS * cw) / 512
                    ready[u] = est["dma"] + 500.0

                def mm_unit(u):
                    _, cw = unit_cols(*u)
                    est["pe"] = max(est["pe"], ready[u]) + (1707.0 * cw) / 512
                    proj_mm(*u)

                kv_all = [(nm, u) for u in range(N_KVU) for nm in ("k", "v")]
                pre = kv_all[:2 * n_b0u]
                rest = kv_all[2 * n_b0u:]
                # units in mm-emission order; DMA order (with the consts
                # woven in by deadline) is built separately below.
                dma_order = [("k", 0), ("v", 0), ("q", 0)] + pre[2:]
                dma_order += [("q", t) for t in range(1, min(5, N_QTT))]
                dma_order += rest
                dma_order += [("q", t) for t in range(5, N_QTT)]

                dma_pend = list(dma_order)
                mm_pend = list(dma_order)
                y_pend = []

                def covered_units(groups):
                    need = []
                    for g in groups:
                        x = g * 128
                        for u, (c0, cw) in enumerate(KVU):
                            if c0 <= x < c0 + cw:
                                for nm in ("k", "v"):
                                    un = (nm, u)
                                    if un in mm_pend and un not in need:
                                        need.append(un)
                    return need

                # startup DMA sequence, strictly in deadline order on the
                # single scalar queue: each weight/const lands just before
                # its first consumer, each stg tile as early as possible.
                np_pre = len(pre) + 1
                stg_pre, ci = dma_pend[:np_pre], 0

                def const_dma(dst, src):
                    if rep == 0:
                        nc.scalar.dma_start(dst[:], src[:])
                        est["dma"] += 200.0

                const_dma(w_sb["k"], w_src["k"])
                est["dma"] += 550.0
                issue_dma(stg_pre[0])                     # k0
                const_dma(w_sb["v"], w_src["v"])
                est["dma"] += 550.0
                issue_dma(stg_pre[1])                     # v0
                issue_dma(stg_pre[2])                     # q0
                const_dma(mb_sb, mbd)
                for nm, bt in b_sb.items():
                    const_dma(bt, bias_d[nm])
                const_dma(w_sb["q"], w_src["q"])
                est["dma"] += 550.0
                if ident_sb is not None:
                    const_dma(ident_sb, identd)
                if len(stg_pre) > 3:
                    issue_dma(stg_pre[3])                 # k1
                const_dma(ones_sb, onesd)
                for u in stg_pre[4:]:
                    issue_dma(u)                          # v1, k2, v2, ...
                del dma_pend[:np_pre]
                # load the ACT exp table during the idle startup window
                if rep == 0:
                    nc.scalar.activation(scr[0:1, 0:1], mb_sb[0:1, 0:1],
                                         AF.Exp, scale=1.0)
                for u in [("k", 0), ("v", 0), ("q", 0)]:
                    mm_unit(u)
                    mm_pend.remove(u)
                const_dma(wo_sb, wo)

                tiles = [(b, qt) for b in range(B) for qt in range(N_QT)]
                last_ti = len(tiles) - 1
                # the last tile runs as two half-width query windows so its
                # normalize + output drain overlaps the second half's
                # attention instead of sitting entirely in the tail.
                HALVE_LAST = OPTS["halve_last"]
                subtiles = []
                for ti in range(len(tiles)):
                    b, qt = tiles[ti]
                    if ti == last_ti and HALVE_LAST:
                        subtiles.append((ti, b, qt, 0, 256))
                        subtiles.append((ti, b, qt, 256, 256))
                    else:
                        subtiles.append((ti, b, qt, 0, 512))

                def run_subtile(ti, b, qt, qo, qw, final):
                    for cost, need, at_norm in attn_steps(b, qt, qo, qw):
                        est["pe"] += cost
                        while dma_pend and est["dma"] < est["pe"] + LEAD:
                            issue_dma(dma_pend.pop(0))
                        budget = 1800.0
                        # forced: K/V units the next scores depend on, and
                        # the next tile's q projection by its deadline.
                        forced = covered_units(need)
                        if at_norm and ("q", ti + 1) in mm_pend:
                            forced.append(("q", ti + 1))
                        for u in forced:
                            while u in dma_pend:  # must be issued by now
                                issue_dma(dma_pend.pop(0))
                            mm_unit(u)
                            mm_pend.remove(u)
                            budget -= 1707
                        if at_norm and final:
                            # final normalize: nothing else will fill the
                            # PE while the DVE reciprocals run - drain all
                            # ready y units here (copies on ACT: exp done)
                            for dc, c0, cw in y_pend:
                                y_unit(dc, c0, cw, tail=True)
                            del y_pend[:]
                            continue
                        # optional fillers: proj mms whose data has landed,
                        # then ready y units (max 2: deeper bursts stall on
                        # the 2-buffer PSUM pool rotation).
                        y_n = 0
                        while budget > 0:
                            pick = None
                            for u in mm_pend[:2]:
                                if (u in ready
                                        and ready[u] <= est["pe"] + 400
                                        and budget >= 1707):
                                    pick = u
                                    break
                            if pick is not None:
                                mm_unit(pick)
                                mm_pend.remove(pick)
                                budget -= 1707
                            elif (y_pend
                                    and y_n < 2
                                    and budget >= (250 * y_pend[0][2]) // 512):
                                dc, c0, cw = y_pend.pop(0)
                                y_unit(dc, c0, cw)
                                cost_y = (250 * cw) // 512
                                est["pe"] += cost_y
                                budget -= cost_y
                                y_n += 1
                            else:
                                break

                for si, (ti, b, qt, qo, qw) in enumerate(subtiles):
                    if qo == 0:
                        assert ("q", ti) not in mm_pend, f"q{ti} not emitted"
                    run_subtile(ti, b, qt, qo, qw,
                                final=(si == len(subtiles) - 1))
                    # this query window is normalized -> its y units are
                    # ready (full tiles pair up across odd ti for one DMA)
                    c0 = ti * 512 + qo
                    pair = OPTS["y_mode"].startswith("pairs")
                    if qw == 512 and pair:
                        if ti % 2 == 1 and ti < 6:
                            y_pend.extend((dc, c0 - 512, 1024)
                                          for dc in range(N_DC))
                        elif ti >= 6:
                            y_pend.extend((dc, c0, 512)
                                          for dc in range(N_DC))
                    else:
                        y_pend.extend((dc, c0, qw) for dc in range(N_DC))
                for u in list(mm_pend):
                    while u in dma_pend:
                        issue_dma(dma_pend.pop(0))
                    mm_unit(u)
                    mm_pend.remove(u)
                for dc, c0, cw in y_pend:
                    y_unit(dc, c0, cw, tail=True)

    nc.compile()
    return nc


def _host_prep(q, k, v, mask, Wq, bq, Wk, bk, Wv, bv, Wo):
    """Build the per-core input maps. Compacts masked keys out of k/v."""
    import ml_dtypes
    f32 = np.float32
    bf16 = ml_dtypes.bfloat16

    qT = np.ascontiguousarray(q.reshape(T, D).T.astype(bf16))

    # --- key compaction: keep only unmasked tokens, pad groups to 128 ---
    idxs, biases, nkts = [], [], []
    for b in range(B):
        idx = np.flatnonzero(~mask[b])
        nkt = max(1, (len(idx) + 127) // 128)
        pad = 128 * nkt - len(idx)
        bias = np.concatenate([np.zeros(len(idx), f32),
                               np.full(pad, MASK_BIAS, f32)])
        idx = np.concatenate([idx, np.zeros(pad, np.int64)])
        idxs.append(idx)
        biases.append(bias)
        nkts.append(nkt)
    NG = sum(nkts)
    TK = 128 * NG
    TKP = 512 * ((TK + 511) // 512)
    kc = np.concatenate([k[b][idxs[b]] for b in range(B)], axis=0)
    vc = np.concatenate([v[b][idxs[b]] for b in range(B)], axis=0)
    kc = np.concatenate([kc, np.zeros((TKP - TK, D), kc.dtype)], axis=0)
    vc = np.concatenate([vc, np.zeros((TKP - TK, D), vc.dtype)], axis=0)
    kT = np.ascontiguousarray(kc.T.astype(bf16))
    vT = np.ascontiguousarray(vc.T.astype(bf16))
    mb = np.concatenate(biases).reshape(NG, 128).T
    mb = np.ascontiguousarray(mb.astype(f32))
    ones128 = np.ones((128, 128), bf16)
    ident = np.eye(128, dtype=f32)

    def chunked(wT):
        # [D, E] -> [128, N_DC*E]: w[p, dc*E + e] = wT[dc*128 + p, e]
        return np.ascontiguousarray(
            wT.reshape(N_DC, 128, E).transpose(1, 0, 2).reshape(128, D))

    in_maps = []
    for c in range(N_CORES):
        sl = slice(c * E, (c + 1) * E)
        m = {
            "qT": qT, "kT": kT, "vT": vT,
            "wq": chunked(Wq[sl, :].T.astype(bf16)),
            "wk": chunked(Wk[sl, :].T.astype(bf16)),
            "wv": chunked(Wv[sl, :].T.astype(bf16)),
            "wo": np.ascontiguousarray(Wo[:, sl].T.astype(bf16)),
            "mb": mb, "ones128": ones128, "ident": ident,
        }
        if np.any(bq):
            m["bq"] = np.ascontiguousarray(bq[sl].astype(f32).reshape(128, 1))
        if np.any(bk):
            m["bk"] = np.ascontiguousarray(bk[sl].astype(f32).reshape(128, 1))
        if np.any(bv):
            m["bv"] = np.ascontiguousarray(bv[sl].astype(f32).reshape(2, 64).T)
        in_maps.append(m)
    return in_maps, (nkts[0], nkts[1])


def _make_timed_runner(nc, in_maps):
    """Build a reusable jitted runner for `nc` (no output donation — the
    program writes every output element, so uninit result buffers are fine).
    Returns (run_once() -> per-core outputs as numpy, time_iters(n) -> [sec])."""
    import jax
    import time
    import concourse.mybir as mybir
    from concourse import bass2jax
    from jax.experimental.shard_map import shard_map
    from jax.sharding import Mesh, NamedSharding, PartitionSpec

    bass2jax.install_neuronx_cc_hook()

    partition_name = nc.partition_id_tensor.name if nc.partition_id_tensor else None
    in_names, out_names, out_avals, zero_outs = [], [], [], []
    for alloc in nc.m.functions[0].allocations:
        if not isinstance(alloc, mybir.MemoryLocationSet):
            continue
        name = alloc.memorylocations[0].name
        if alloc.kind == "ExternalInput":
            if name != partition_name:
                in_names.append(name)
        elif alloc.kind == "ExternalOutput":
            shape = tuple(alloc.tensor_shape)
            dtype = mybir.dt.np(alloc.dtype)
            out_names.append(name)
            out_avals.append(jax.core.ShapedArray(shape, dtype))
            zero_outs.append(np.zeros(shape, dtype))
    n_params = len(in_names)
    all_in_names = list(in_names) + list(out_names)
    if partition_name is not None:
        all_in_names.append(partition_name)

    def _body(*args):
        operands = list(args)
        if partition_name is not None:
            operands.append(bass2jax.partition_id_tensor())
        outs = bass2jax._bass_exec_p.bind(
            *operands,
            out_avals=tuple(out_avals),
            in_names=tuple(all_in_names),
            out_names=tuple(out_names),
            lowering_input_output_aliases=(),
            sim_require_finite=True,
            sim_require_nnan=True,
            nc=nc,
        )
        return tuple(outs)

    devices = jax.devices()[:N_CORES]
    mesh = Mesh(np.asarray(devices), ("core",))
    nin = n_params + len(out_names)
    fn = jax.jit(shard_map(_body, mesh=mesh,
                           in_specs=(PartitionSpec("core"),) * nin,
                           out_specs=(PartitionSpec("core"),) * len(out_names),
                           check_rep=False))
    sh = NamedSharding(mesh, PartitionSpec("core"))
    dev_args = [
        jax.device_put(
            np.concatenate([np.asarray(in_maps[c][nm]) for c in range(N_CORES)],
                           axis=0), sh)
        for nm in in_names
    ] + [
        jax.device_put(np.zeros((N_CORES * z.shape[0], *z.shape[1:]), z.dtype), sh)
        for z in zero_outs
    ]

    def run_once():
        outs = fn(*dev_args)
        jax.block_until_ready(outs)
        return [
            {nm: np.asarray(outs[i]).reshape(N_CORES, *out_avals[i].shape)[c]
             for i, nm in enumerate(out_names)}
            for c in range(N_CORES)
        ]

    def time_iters(n):
        ts = []
        for _ in range(n):
            t0 = time.perf_counter()
            jax.block_until_ready(fn(*dev_args))
            ts.append(time.perf_counter() - t0)
        return ts

    return run_once, time_iters


def kernel(q, k, v, mask, Wq, bq, Wk, bk, Wv, bv, Wo, bo):
    from concourse.bass_utils import run_bass_kernel_spmd

    q, k, v = (np.asarray(x) for x in (q, k, v))
    mask = np.asarray(mask)
    in_maps, nkt = _host_prep(q, k, v, mask, np.asarray(Wq), np.asarray(bq),
                              np.asarray(Wk), np.asarray(bk), np.asarray(Wv),
                              np.asarray(bv), np.asarray(Wo))
    key = (("bq" in in_maps[0]), ("bk" in in_maps[0]), ("bv" in in_maps[0]),
           nkt[0], nkt[1])
    if key not in _cached:
        _cached[key] = _build_program(*key)
    nc = _cached[key]

    trace = bool(int(os.environ.get("KERNEL_TRACE", "0")))
    res = run_bass_kernel_spmd(nc, in_maps, list(range(N_CORES)), trace=trace)
    kernel.last_results = res

    yT = np.zeros((D, T), np.float32)
    for i in range(N_CORES):
        yT += res.results[i]["y"].astype(np.float32)
    y = yT.T + np.asarray(bo).astype(np.float32)
    return np.ascontiguousarray(y.astype(np.float32)).reshape(B, L, D)
